# revision 10
# baseline (speedup 1.0000x reference)
"""Trainium2 Bass kernel for nn_Better_Transformer (block-diag MLP + BatchNorm + tanh ×2).

  o1 = tanh(BN(x @ blockdiag(w1) + b1))
  o3 = tanh(BN(o1 @ blockdiag(w2) + b2 + x))

Strategy (8 NeuronCores, data-parallel over the batch dim):
  - Each core owns 2048 of the 16384 rows; weights/BN params replicated.
  - Feature-major layout on chip ([128 features, rows]): BatchNorm
    reductions are free-dim reductions and matmuls stream rows as the
    moving operand (weights stationary), N=1024 bf16 moving tiles.
  - Host pre-transposes x to [F, B/8] bf16 per core; output returns
    feature-major bf16 and the host transposes/upcasts back.
  - bias1/bias2 cancel inside BatchNorm and never reach the device.
  - BN statistics: per-core (mean, E[y²]) per feature → 32 KB AllGather
    over the 8 cores → local reduce → global mean/var.  Stage-A stats
    are split between VectorE (bn_stats) and ScalarE (Copy/Square with
    accum_out) so both engines share the scan.
  - The residual (+x) is accumulated on the TensorEngine via an
    identity-matrix matmul into the same PSUM group as matmul2.
  - BN affine + tanh fuse into one ScalarEngine activation per tile
    (per-partition scale/bias APs).
  - y1 is recomputed in stage B instead of stored; u = o2+x overwrites
    the resident x blockwise (one 16 MB SBUF region holds x then u).
  - A warm-up burst of matmuls trips the PE HAM throttle to 2.4 GHz
    while the input DMAs are still in flight.
"""

import os
import sys
import types

import numpy as np
import ml_dtypes

B, F, P, D = 16384, 4096, 32, 128
NCORES = 8
BC = B // NCORES          # 2048 rows per core
NW = 1024                 # matmul moving-dim (bf16 allows 1024)
NH = BC // NW             # 2 wide chunks per block row-range
EPS = 1e-5

# Stage-A engine split: these blocks' stats run on ScalarE (accum_out),
# the rest on VectorE (bn_stats).  ~13/32 balances 2×FD1024 ACT ops
# against 4×FD512 bn_stats.
ACT_BLOCKS = [0, 3, 6, 9, 12, 15, 18, 21, 24, 27, 30]
DVE_BLOCKS = [p for p in range(P) if p not in ACT_BLOCKS]
GROUPED = DVE_BLOCKS + ACT_BLOCKS           # payload column order for sync 1
COL1 = {p: i for i, p in enumerate(GROUPED)}

_BF16 = ml_dtypes.bfloat16

_state: dict = {}


def _install_ldw_opt_patch():
    """bass hardcodes --enable-ldw-opt=false; walrus's own default is
    true.  Re-enable it (BASS_LDW_OPT=0 reverts) so repeated-lhsT matmul
    runs don't reload the PE weight array every instruction."""
    if _state.get("ldw_patched") or os.environ.get("BASS_LDW_OPT", "0") != "1":
        return
    _state["ldw_patched"] = True
    import concourse.bass_utils as bu
    real = bu.run_command

    def wrapper(argv, **kw):
        argv = ["--enable-ldw-opt=true" if a == "--enable-ldw-opt=false" else a
                for a in argv]
        return real(argv, **kw)

    bu.run_command = wrapper


def _install_tile_drain_patch():
    """This walrus build rejects >1 sem wait per instruction ("Too many
    sync wait commands" in setupSyncWait).  1) split the end-of-kernel
    drain waits across single-wait NOPs; 2) after assign_waits, hoist
    extra per-instruction waits onto nofuse NOPs."""
    if _state.get("patched"):
        return
    _state["patched"] = True
    import concourse.mybir as mybir
    import concourse.tile as tile_mod
    from concourse.tile import TileContext
    from concourse.vector_clock import ScopedClock, VectorClock

    def _drain_and_barrier(self, tick_clock, wait_clock):
        gc = tick_clock.global_clock
        for i in range(len(gc)):
            if gc[i] > 0:
                c = VectorClock()
                c.require_at_least(i, gc[i])
                nop = self.nc.sync.nop(nofuse=True, hint="tile_exit_wait")
                wait_clock.add_sem_waits(nop.ins, ScopedClock({None: c}))
        self.nc.sync.drain()
        self.nc.all_engine_barrier()
        assert self.sems is not None
        popped = self.nc._tile_sem_poison_stack.pop()
        assert popped is self._sem_poison
        self.nc.clear_and_free_semaphores(list(self.sems.allocated().values()))
        self.nc.all_engine_barrier()

    TileContext._drain_and_barrier = _drain_and_barrier

    _RealWait = tile_mod.TileClockWait

    class _WaitSplitClockWait:
        def __init__(self, tc, ordered):
            self._w = _RealWait(tc, ordered)
            self._tc = tc
            self._ordered = ordered

        def assign_waits(self, bb_name):
            r = self._w.assign_waits(bb_name)
            nc = self._tc.nc
            for insts in self._ordered.values():
                out = []
                for inst in insts:
                    si = inst.sync_info
                    if si is not None and si.on_wait and len(si.on_wait) > 1:
                        waits = list(si.on_wait)
                        for w in waits[:-1]:
                            nop = mybir.InstNoOp(
                                name=nc.get_next_instruction_name(),
                                engine=inst.engine, ins=[], outs=[],
                            )
                            nop.bass_nofuse = True
                            nop.sync_info = mybir.SyncInfo(on_wait=[w], on_update=[])
                            out.append(nop)
                        si.on_wait = [waits[-1]]
                    out.append(inst)
                insts[:] = out
            return r

        def __getattr__(self, k):
            return getattr(self._w, k)

    tile_mod.TileClockWait = _WaitSplitClockWait


def _install_ntff_hook():
    """Optional: lets BASS_TRACE=1 produce an NTFF profile under axon when
    the image's antenv lacks axon_hooks.  Safe no-op on any failure."""
    if "antenv.axon_hooks" in sys.modules:
        return
    try:
        import contextlib
        import ctypes

        so_path = "/opt/axon/libaxon_pjrt.so"
        if not os.path.exists(so_path):
            return
        lib = ctypes.CDLL(so_path)
        if not hasattr(lib, "axon_start_nrt_profile"):
            return
        lib.axon_start_nrt_profile.argtypes = [ctypes.POINTER(ctypes.c_int64), ctypes.c_size_t]
        lib.axon_start_nrt_profile.restype = ctypes.c_int64
        lib.axon_stop_nrt_profile.argtypes = [ctypes.c_char_p]
        lib.axon_stop_nrt_profile.restype = ctypes.c_int64

        @contextlib.contextmanager
        def _hook(output_dir, device_ids):
            import jax
            jax.devices()
            if device_ids:
                ids = (ctypes.c_int64 * len(device_ids))(*device_ids)
                rc = lib.axon_start_nrt_profile(ids, len(device_ids))
            else:
                rc = lib.axon_start_nrt_profile(None, 0)
            if rc != 0:
                raise RuntimeError(f"axon_start_nrt_profile rc={rc}")
            try:
                yield
            finally:
                n = lib.axon_stop_nrt_profile(str(output_dir).encode())
                if n <= 0:
                    print(f"ntff profile: {n} files written", file=sys.stderr)

        mod = types.ModuleType("antenv.axon_hooks")
        mod.get_axon_ntff_profile_hook = lambda: _hook
        mod.set_axon_ntff_profile_hook = lambda h: None
        sys.modules["antenv.axon_hooks"] = mod
    except Exception:
        pass


def _build():
    import concourse.bass as bass
    import concourse.mybir as mybir
    import concourse.tile as tile

    f32 = mybir.dt.float32
    bf16 = mybir.dt.bfloat16
    Tanh = mybir.ActivationFunctionType.Tanh
    Sqrt = mybir.ActivationFunctionType.Sqrt
    Copy = mybir.ActivationFunctionType.Copy
    Square = mybir.ActivationFunctionType.Square
    mult = mybir.AluOpType.mult
    add = mybir.AluOpType.add
    subtract = mybir.AluOpType.subtract
    AX = mybir.AxisListType.X

    nc = bass.Bass(trn_type="TRN2", num_devices=NCORES)

    xt = nc.dram_tensor("xt", [F, BC], bf16, kind="ExternalInput")
    w1 = nc.dram_tensor("w1", [D, F], bf16, kind="ExternalInput")
    w2 = nc.dram_tensor("w2", [D, F], bf16, kind="ExternalInput")
    ident = nc.dram_tensor("ident", [D, D], bf16, kind="ExternalInput")
    g1 = nc.dram_tensor("g1", [D, P], f32, kind="ExternalInput")   # grouped col order
    bt1 = nc.dram_tensor("bt1", [D, P], f32, kind="ExternalInput")  # grouped col order
    g3 = nc.dram_tensor("g3", [D, P], f32, kind="ExternalInput")   # natural order
    bt3 = nc.dram_tensor("bt3", [D, P], f32, kind="ExternalInput")
    out = nc.dram_tensor("out", [F, BC], bf16, kind="ExternalOutput")

    n_act = len(ACT_BLOCKS)
    n_dve = len(DVE_BLOCKS)

    with tile.TileContext(nc) as tc:
        with (
            tc.tile_pool(name="const", bufs=1) as const,
            tc.tile_pool(name="xup", bufs=1) as xup,
            tc.tile_pool(name="stat", bufs=1) as statp,
            tc.tile_pool(name="o1p", bufs=2) as o1p,
            tc.tile_pool(name="scrp", bufs=2) as scrp,
            tc.tile_pool(name="ofp", bufs=4) as ofp,
            tc.tile_pool(name="psa", bufs=2, space="PSUM") as psa,
            tc.tile_pool(name="psb", bufs=2, space="PSUM") as psb,
            tc.tile_pool(name="dram", bufs=1, space="DRAM") as dram,
        ):
            # ncfw warm-up: a tiny AllGather so the first real collective
            # doesn't pay the cold trigger-start delay.  Overlaps the xt DMAs.
            wu_sb = statp.tile([D, 2], f32)
            nc.vector.memset(wu_sb, 0.0)
            wu_in = dram.tile([D, 2], f32, tag="wu_in")
            wu_out = dram.tile([NCORES * D, 2], f32, tag="wu_out")
            nc.sync.dma_start(wu_in, wu_sb)
            nc.gpsimd.collective_compute(
                "AllGather", mybir.AluOpType.bypass,
                replica_groups=[list(range(NCORES))],
                ins=[wu_in.opt()], outs=[wu_out.opt()],
            )

            w1_sb = const.tile([D, F], bf16)
            w2_sb = const.tile([D, F], bf16)
            id_sb = const.tile([D, D], bf16)
            g1_sb = const.tile([D, P], f32)
            bt1_sb = const.tile([D, P], f32)
            g3_sb = const.tile([D, P], f32)
            bt3_sb = const.tile([D, P], f32)
            nc.sync.dma_start(w1_sb, w1[:])
            nc.sync.dma_start(w2_sb, w2[:])
            nc.sync.dma_start(id_sb, ident[:])
            nc.sync.dma_start(g1_sb, g1[:])
            nc.sync.dma_start(bt1_sb, bt1[:])
            nc.sync.dma_start(g3_sb, g3[:])
            nc.sync.dma_start(bt3_sb, bt3[:])

            # PE HAM warm-up: a dense burst of matmuls on the (tiny) w1
            # tile while the big xt DMAs stream in.
            for i in range(24):
                pw = psa.tile([D, NW], f32, tag="pp")
                nc.tensor.matmul(pw[:, 0:NW // 2], lhsT=w1_sb[:, 0:D],
                                 rhs=w1_sb[:, 0:NW // 2], start=True, stop=True)
                nc.tensor.matmul(pw[:, NW // 2:NW], lhsT=w1_sb[:, 0:D],
                                 rhs=w1_sb[:, NW // 2:NW], start=True, stop=True)

            xu = []
            for p in range(P):
                t = xup.tile([D, BC], bf16, tag=f"xu{p}")
                nc.sync.dma_start(t, xt[p * D:(p + 1) * D, :])
                xu.append(t)

            stats1 = statp.tile([D, n_dve, 4, 6], f32)   # DVE blocks, 512-wide
            stats2 = statp.tile([D, P, 4, 6], f32)
            mv1 = statp.tile([D, n_dve, 2], f32)
            mv2 = statp.tile([D, P, 2], f32)
            sa = statp.tile([D, n_act, 2], f32)          # ACT-block sums
            qa = statp.tile([D, n_act, 2], f32)          # ACT-block sumsqs
            arpay1 = statp.tile([D, 2 * P], f32)
            arpay2 = statp.tile([D, 2 * P], f32)
            red1 = statp.tile([D, 2 * P], f32)
            red2 = statp.tile([D, 2 * P], f32)
            gath1 = statp.tile([D, NCORES, 2 * P], f32)
            gath2 = statp.tile([D, NCORES, 2 * P], f32)
            Mt = statp.tile([D, P], f32)
            Qt = statp.tile([D, P], f32)
            vt = statp.tile([D, P], f32)
            s1 = statp.tile([D, P], f32)
            t1 = statp.tile([D, P], f32)
            s3 = statp.tile([D, P], f32)
            t3 = statp.tile([D, P], f32)
            eps_sb = statp.tile([D, 1], f32)
            nc.vector.memset(eps_sb, EPS)

            def wcol(w_sb, p):
                return w_sb[:, p * D:(p + 1) * D]

            def all_gather(arpay, gath, red, tagn):
                agin = dram.tile([D, 2 * P], f32, tag=f"agin{tagn}")
                agout = dram.tile([NCORES * D, 2 * P], f32, tag=f"agout{tagn}")
                nc.sync.dma_start(agin, arpay)
                nc.gpsimd.collective_compute(
                    "AllGather", mybir.AluOpType.bypass,
                    replica_groups=[list(range(NCORES))],
                    ins=[agin.opt()], outs=[agout.opt()],
                )
                nc.sync.dma_start(gath, agout.rearrange("(r i) f -> i r f", r=NCORES))
                nc.vector.tensor_reduce(out=red, in_=gath[:].rearrange("i r f -> i f r"),
                                        axis=AX, op=add)

            def affine(red, g_sb, b_sb, s, t):
                # red[:, 0:P] = Σ_cores mean ; red[:, P:2P] = Σ_cores E[y²]
                nc.vector.tensor_scalar_mul(Mt, red[:, 0:P], 1.0 / NCORES)
                nc.vector.tensor_scalar_mul(Qt, red[:, P:2 * P], 1.0 / NCORES)
                nc.vector.tensor_tensor(vt, Mt, Mt, op=mult)
                nc.vector.tensor_tensor(vt, Qt, vt, op=subtract)          # global var
                nc.scalar.activation(out=vt, in_=vt, func=Sqrt, bias=eps_sb)
                nc.vector.reciprocal(vt, vt)                              # rstd
                nc.vector.tensor_tensor(s, g_sb, vt, op=mult)
                nc.vector.tensor_tensor(t, Mt, s, op=mult)
                nc.vector.tensor_tensor(t, b_sb, t, op=subtract)          # beta - M*s

            # ---- Stage A: per-core stats of y1 = x @ W1 ----
            for p in range(P):
                j = None
                if p in ACT_BLOCKS:
                    j = ACT_BLOCKS.index(p)
                else:
                    j = DVE_BLOCKS.index(p)
                pool = psa if p % 2 == 0 else psb
                for h in range(NH):
                    ps = pool.tile([D, NW], f32, tag="pp" if pool is psa else "qq")
                    for q in range(2):
                        qs = slice(q * (NW // 2), (q + 1) * (NW // 2))
                        nc.tensor.matmul(ps[:, qs], lhsT=wcol(w1_sb, p),
                                         rhs=xu[p][:, h * NW + q * (NW // 2):
                                                   h * NW + (q + 1) * (NW // 2)],
                                         start=True, stop=True)
                    if p in ACT_BLOCKS:
                        scr = scrp.tile([D, NW], bf16, tag="scr")
                        nc.scalar.activation(out=scr, in_=ps, func=Copy,
                                             accum_out=sa[:, j, h:h + 1])
                        nc.scalar.activation(out=scr, in_=ps, func=Square,
                                             accum_out=qa[:, j, h:h + 1])
                    else:
                        nc.vector.bn_stats(out=stats1[:, j, 2 * h], in_=ps[:, 0:NW // 2])
                        nc.vector.bn_stats(out=stats1[:, j, 2 * h + 1], in_=ps[:, NW // 2:NW])
                if p not in ACT_BLOCKS:
                    nc.vector.bn_aggr(out=mv1[:, j], in_=stats1[:, j])

            # payload (grouped column order: DVE blocks then ACT blocks)
            nc.vector.tensor_copy(arpay1[:, 0:n_dve], mv1[:, :, 0])
            nc.vector.tensor_tensor(arpay1[:, P:P + n_dve], mv1[:, :, 0], mv1[:, :, 0], op=mult)
            nc.vector.tensor_tensor(arpay1[:, P:P + n_dve], arpay1[:, P:P + n_dve],
                                    mv1[:, :, 1], op=add)
            nc.vector.tensor_reduce(out=arpay1[:, n_dve:P], in_=sa[:], axis=AX, op=add)
            nc.vector.tensor_reduce(out=arpay1[:, P + n_dve:2 * P], in_=qa[:], axis=AX, op=add)
            nc.vector.tensor_scalar_mul(arpay1[:, n_dve:P], arpay1[:, n_dve:P], 1.0 / BC)
            nc.vector.tensor_scalar_mul(arpay1[:, P + n_dve:2 * P],
                                        arpay1[:, P + n_dve:2 * P], 1.0 / BC)

            all_gather(arpay1, gath1, red1, 1)
            # keep the PE HAM warm through the collective gap (slot reuse of
            # the "pp" pool orders these after stage A's matmuls)
            for i in range(20):
                pw = psa.tile([D, NW], f32, tag="pp")
                nc.tensor.matmul(pw[:, 0:NW // 2], lhsT=w1_sb[:, 0:D],
                                 rhs=w1_sb[:, 0:NW // 2], start=True, stop=True)
                nc.tensor.matmul(pw[:, NW // 2:NW], lhsT=w1_sb[:, 0:D],
                                 rhs=w1_sb[:, NW // 2:NW], start=True, stop=True)
            affine(red1, g1_sb, bt1_sb, s1, t1)   # grouped col order

            # ---- Stage B: o1 = tanh(s1·y1 + t1); u = o1 @ W2 + x ----
            for p in range(P):
                c1 = COL1[p]
                o1 = o1p.tile([D, BC], bf16, tag="o1")
                pss = []
                for h in range(NH):
                    ps = psa.tile([D, NW], f32, tag="pp")
                    pss.append(ps)
                    for q in range(2):
                        nc.tensor.matmul(ps[:, q * (NW // 2):(q + 1) * (NW // 2)],
                                         lhsT=wcol(w1_sb, p),
                                         rhs=xu[p][:, h * NW + q * (NW // 2):
                                                   h * NW + (q + 1) * (NW // 2)],
                                         start=True, stop=True)
                for h in range(NH):
                    hs = slice(h * NW, (h + 1) * NW)
                    nc.scalar.activation(out=o1[:, hs], in_=pss[h], func=Tanh,
                                         bias=t1[:, c1:c1 + 1], scale=s1[:, c1:c1 + 1])
                # one LDW of W2 for all four halves, then one LDW of identity
                pus = [psb.tile([D, NW], f32, tag="qq", name=f"pu{h}") for h in range(NH)]
                for h in range(NH):
                    for q in range(2):
                        gsl = slice(h * NW + q * (NW // 2), h * NW + (q + 1) * (NW // 2))
                        nc.tensor.matmul(pus[h][:, q * (NW // 2):(q + 1) * (NW // 2)],
                                         lhsT=wcol(w2_sb, p), rhs=o1[:, gsl],
                                         start=True, stop=False)
                for h in range(NH):
                    for q in range(2):
                        gsl = slice(h * NW + q * (NW // 2), h * NW + (q + 1) * (NW // 2))
                        nc.tensor.matmul(pus[h][:, q * (NW // 2):(q + 1) * (NW // 2)],
                                         lhsT=id_sb, rhs=xu[p][:, gsl],
                                         start=False, stop=True)
                for h in range(NH):
                    hs = slice(h * NW, (h + 1) * NW)
                    if p % 3 == 2:
                        nc.vector.tensor_copy(out=xu[p][:, hs], in_=pus[h])
                    else:
                        nc.scalar.activation(out=xu[p][:, hs], in_=pus[h],
                                             func=Copy)   # u overwrites x
                    nc.vector.bn_stats(out=stats2[:, p, 2 * h],
                                       in_=xu[p][:, h * NW:h * NW + NW // 2])
                    nc.vector.bn_stats(out=stats2[:, p, 2 * h + 1],
                                       in_=xu[p][:, h * NW + NW // 2:(h + 1) * NW])
                nc.vector.bn_aggr(out=mv2[:, p], in_=stats2[:, p])

            nc.vector.tensor_copy(arpay2[:, 0:P], mv2[:, :, 0])
            nc.vector.tensor_tensor(arpay2[:, P:2 * P], mv2[:, :, 0], mv2[:, :, 0], op=mult)
            nc.vector.tensor_tensor(arpay2[:, P:2 * P], arpay2[:, P:2 * P],
                                    mv2[:, :, 1], op=add)

            all_gather(arpay2, gath2, red2, 2)
            affine(red2, g3_sb, bt3_sb, s3, t3)   # natural col order

            # ---- Stage C: out = tanh(s3·u + t3) ----
            for p in range(P):
                of = ofp.tile([D, BC], bf16, tag="of")
                nc.scalar.activation(out=of, in_=xu[p], func=Tanh,
                                     bias=t3[:, p:p + 1], scale=s3[:, p:p + 1])
                nc.sync.dma_start(out[p * D:(p + 1) * D, :], of)

    return nc


def _get_nc():
    if "nc" not in _state:
        _install_tile_drain_patch()
        _install_ldw_opt_patch()
        _install_ntff_hook()
        _state["nc"] = _build()
    return _state["nc"]


def kernel(x, weights1, bias1, weights2, bias2, gamma1, beta1, gamma3, beta3):
    from concourse.bass_utils import run_bass_kernel_spmd

    x = np.asarray(x, dtype=np.float32)
    w1 = np.asarray(weights1, dtype=np.float32)
    w2 = np.asarray(weights2, dtype=np.float32)
    gamma1 = np.asarray(gamma1, dtype=np.float32)
    beta1 = np.asarray(beta1, dtype=np.float32)
    gamma3 = np.asarray(gamma3, dtype=np.float32)
    beta3 = np.asarray(beta3, dtype=np.float32)

    nc = _get_nc()

    xT = np.ascontiguousarray(x.T).astype(_BF16)            # [F, B]
    w1h = np.ascontiguousarray(w1.transpose(1, 0, 2).reshape(D, F)).astype(_BF16)
    w2h = np.ascontiguousarray(w2.transpose(1, 0, 2).reshape(D, F)).astype(_BF16)
    identh = np.eye(D, dtype=np.float32).astype(_BF16)
    perm = np.asarray(GROUPED)
    g1h = np.ascontiguousarray(gamma1.reshape(P, D).T[:, perm])
    bt1h = np.ascontiguousarray(beta1.reshape(P, D).T[:, perm])
    g3h = np.ascontiguousarray(gamma3.reshape(P, D).T)
    bt3h = np.ascontiguousarray(beta3.reshape(P, D).T)

    in_maps = []
    for cid in range(NCORES):
        in_maps.append({
            "xt": np.ascontiguousarray(xT[:, cid * BC:(cid + 1) * BC]),
            "w1": w1h, "w2": w2h, "ident": identh,
            "g1": g1h, "bt1": bt1h, "g3": g3h, "bt3": bt3h,
        })

    res = run_bass_kernel_spmd(nc, in_maps, core_ids=list(range(NCORES)))
    _state["last_exec_time_ns"] = res.exec_time_ns

    outT = np.empty((B, F), dtype=np.float32)
    for cid in range(NCORES):
        outT[cid * BC:(cid + 1) * BC, :] = res.results[cid]["out"].T.astype(np.float32)
    return outT


# revision 11
# speedup vs baseline: 2.1186x; 2.1186x over previous
"""Trainium2 Bass kernel for nn_Better_Transformer (block-diag MLP + BatchNorm + tanh ×2).

  o1 = tanh(BN(x @ blockdiag(w1) + b1))
  o3 = tanh(BN(o1 @ blockdiag(w2) + b2 + x))

Strategy (8 NeuronCores, data-parallel over the batch dim):
  - Each core owns 2048 of the 16384 rows; weights/BN params replicated.
  - Feature-major layout on chip ([128 features, rows]): BatchNorm
    reductions are free-dim reductions and matmuls stream rows as the
    moving operand (weights stationary), N=1024 bf16 moving tiles.
  - Host pre-transposes x to [F, B/8] bf16 per core; output returns
    feature-major bf16 and the host transposes/upcasts back.
  - bias1/bias2 cancel inside BatchNorm and never reach the device.
  - BN statistics: per-core (mean, E[y²]) per feature → 32 KB AllGather
    over the 8 cores → local reduce → global mean/var.  Stage-A stats
    are split between VectorE (bn_stats) and ScalarE (Copy/Square with
    accum_out) so both engines share the scan.
  - The residual (+x) is accumulated on the TensorEngine via an
    identity-matrix matmul into the same PSUM group as matmul2.
  - BN affine + tanh fuse into one ScalarEngine activation per tile
    (per-partition scale/bias APs).
  - y1 is recomputed in stage B instead of stored; u = o2+x overwrites
    the resident x blockwise (one 16 MB SBUF region holds x then u).
  - A warm-up burst of matmuls trips the PE HAM throttle to 2.4 GHz
    while the input DMAs are still in flight.
"""

import os
import sys
import types

import numpy as np
import ml_dtypes

B, F, P, D = 16384, 4096, 32, 128
NCORES = 8
BC = B // NCORES          # 2048 rows per core
NW = 1024                 # matmul moving-dim (bf16 allows 1024)
NH = BC // NW             # 2 wide chunks per block row-range
EPS = 1e-5

# Stage-A engine split: these blocks' stats run on ScalarE (accum_out),
# the rest on VectorE (bn_stats).  ~13/32 balances 2×FD1024 ACT ops
# against 4×FD512 bn_stats.
ACT_BLOCKS = [0, 3, 6, 9, 12, 15, 18, 21, 24, 27, 30]
DVE_BLOCKS = [p for p in range(P) if p not in ACT_BLOCKS]
GROUPED = DVE_BLOCKS + ACT_BLOCKS           # payload column order for sync 1
COL1 = {p: i for i, p in enumerate(GROUPED)}

_BF16 = ml_dtypes.bfloat16

_state: dict = {}


def _install_ldw_opt_patch():
    """bass hardcodes --enable-ldw-opt=false; walrus's own default is
    true.  Re-enable it (BASS_LDW_OPT=0 reverts) so repeated-lhsT matmul
    runs don't reload the PE weight array every instruction."""
    if _state.get("ldw_patched") or os.environ.get("BASS_LDW_OPT", "0") != "1":
        return
    _state["ldw_patched"] = True
    import concourse.bass_utils as bu
    real = bu.run_command

    def wrapper(argv, **kw):
        argv = ["--enable-ldw-opt=true" if a == "--enable-ldw-opt=false" else a
                for a in argv]
        return real(argv, **kw)

    bu.run_command = wrapper


def _install_tile_drain_patch():
    """This walrus build rejects >1 sem wait per instruction ("Too many
    sync wait commands" in setupSyncWait).  1) split the end-of-kernel
    drain waits across single-wait NOPs; 2) after assign_waits, hoist
    extra per-instruction waits onto nofuse NOPs."""
    if _state.get("patched"):
        return
    _state["patched"] = True
    import concourse.mybir as mybir
    import concourse.tile as tile_mod
    from concourse.tile import TileContext
    from concourse.vector_clock import ScopedClock, VectorClock

    def _drain_and_barrier(self, tick_clock, wait_clock):
        gc = tick_clock.global_clock
        for i in range(len(gc)):
            if gc[i] > 0:
                c = VectorClock()
                c.require_at_least(i, gc[i])
                nop = self.nc.sync.nop(nofuse=True, hint="tile_exit_wait")
                wait_clock.add_sem_waits(nop.ins, ScopedClock({None: c}))
        self.nc.sync.drain()
        self.nc.all_engine_barrier()
        assert self.sems is not None
        popped = self.nc._tile_sem_poison_stack.pop()
        assert popped is self._sem_poison
        self.nc.clear_and_free_semaphores(list(self.sems.allocated().values()))
        self.nc.all_engine_barrier()

    TileContext._drain_and_barrier = _drain_and_barrier

    _RealWait = tile_mod.TileClockWait

    class _WaitSplitClockWait:
        def __init__(self, tc, ordered):
            self._w = _RealWait(tc, ordered)
            self._tc = tc
            self._ordered = ordered

        def assign_waits(self, bb_name):
            r = self._w.assign_waits(bb_name)
            nc = self._tc.nc
            for insts in self._ordered.values():
                out = []
                for inst in insts:
                    si = inst.sync_info
                    if si is not None and si.on_wait and len(si.on_wait) > 1:
                        waits = list(si.on_wait)
                        for w in waits[:-1]:
                            nop = mybir.InstNoOp(
                                name=nc.get_next_instruction_name(),
                                engine=inst.engine, ins=[], outs=[],
                            )
                            nop.bass_nofuse = True
                            nop.sync_info = mybir.SyncInfo(on_wait=[w], on_update=[])
                            out.append(nop)
                        si.on_wait = [waits[-1]]
                    out.append(inst)
                insts[:] = out
            return r

        def __getattr__(self, k):
            return getattr(self._w, k)

    tile_mod.TileClockWait = _WaitSplitClockWait


def _install_ntff_hook():
    """Optional: lets BASS_TRACE=1 produce an NTFF profile under axon when
    the image's antenv lacks axon_hooks.  Safe no-op on any failure."""
    if "antenv.axon_hooks" in sys.modules:
        return
    try:
        import contextlib
        import ctypes

        so_path = "/opt/axon/libaxon_pjrt.so"
        if not os.path.exists(so_path):
            return
        lib = ctypes.CDLL(so_path)
        if not hasattr(lib, "axon_start_nrt_profile"):
            return
        lib.axon_start_nrt_profile.argtypes = [ctypes.POINTER(ctypes.c_int64), ctypes.c_size_t]
        lib.axon_start_nrt_profile.restype = ctypes.c_int64
        lib.axon_stop_nrt_profile.argtypes = [ctypes.c_char_p]
        lib.axon_stop_nrt_profile.restype = ctypes.c_int64

        @contextlib.contextmanager
        def _hook(output_dir, device_ids):
            import jax
            jax.devices()
            if device_ids:
                ids = (ctypes.c_int64 * len(device_ids))(*device_ids)
                rc = lib.axon_start_nrt_profile(ids, len(device_ids))
            else:
                rc = lib.axon_start_nrt_profile(None, 0)
            if rc != 0:
                raise RuntimeError(f"axon_start_nrt_profile rc={rc}")
            try:
                yield
            finally:
                n = lib.axon_stop_nrt_profile(str(output_dir).encode())
                if n <= 0:
                    print(f"ntff profile: {n} files written", file=sys.stderr)

        mod = types.ModuleType("antenv.axon_hooks")
        mod.get_axon_ntff_profile_hook = lambda: _hook
        mod.set_axon_ntff_profile_hook = lambda h: None
        sys.modules["antenv.axon_hooks"] = mod
    except Exception:
        pass


def _build():
    import concourse.bass as bass
    import concourse.mybir as mybir
    import concourse.tile as tile

    f32 = mybir.dt.float32
    bf16 = mybir.dt.bfloat16
    Tanh = mybir.ActivationFunctionType.Tanh
    Sqrt = mybir.ActivationFunctionType.Sqrt
    Copy = mybir.ActivationFunctionType.Copy
    Square = mybir.ActivationFunctionType.Square
    mult = mybir.AluOpType.mult
    add = mybir.AluOpType.add
    subtract = mybir.AluOpType.subtract
    AX = mybir.AxisListType.X

    nc = bass.Bass(trn_type="TRN2", num_devices=NCORES)

    xt = nc.dram_tensor("xt", [F, BC], bf16, kind="ExternalInput")
    w1 = nc.dram_tensor("w1", [D, F], bf16, kind="ExternalInput")
    w2 = nc.dram_tensor("w2", [D, F], bf16, kind="ExternalInput")
    ident = nc.dram_tensor("ident", [D, D], bf16, kind="ExternalInput")
    g1 = nc.dram_tensor("g1", [D, P], f32, kind="ExternalInput")   # grouped col order
    bt1 = nc.dram_tensor("bt1", [D, P], f32, kind="ExternalInput")  # grouped col order
    g3 = nc.dram_tensor("g3", [D, P], f32, kind="ExternalInput")   # natural order
    bt3 = nc.dram_tensor("bt3", [D, P], f32, kind="ExternalInput")
    out = nc.dram_tensor("out", [F, BC], bf16, kind="ExternalOutput")

    n_act = len(ACT_BLOCKS)
    n_dve = len(DVE_BLOCKS)

    with tile.TileContext(nc) as tc:
        with (
            tc.tile_pool(name="const", bufs=1) as const,
            tc.tile_pool(name="xup", bufs=1) as xup,
            tc.tile_pool(name="stat", bufs=1) as statp,
            tc.tile_pool(name="o1p", bufs=2) as o1p,
            tc.tile_pool(name="scrp", bufs=2) as scrp,
            tc.tile_pool(name="ofp", bufs=4) as ofp,
            tc.tile_pool(name="psa", bufs=2, space="PSUM") as psa,
            tc.tile_pool(name="psb", bufs=2, space="PSUM") as psb,
            tc.tile_pool(name="dram", bufs=1, space="DRAM") as dram,
        ):
            w1_sb = const.tile([D, F], bf16)
            w2_sb = const.tile([D, F], bf16)
            id_sb = const.tile([D, D], bf16)
            g1_sb = const.tile([D, P], f32)
            bt1_sb = const.tile([D, P], f32)
            g3_sb = const.tile([D, P], f32)
            bt3_sb = const.tile([D, P], f32)
            nc.sync.dma_start(w1_sb, w1[:])
            nc.sync.dma_start(w2_sb, w2[:])
            nc.sync.dma_start(id_sb, ident[:])
            nc.sync.dma_start(g1_sb, g1[:])
            nc.sync.dma_start(bt1_sb, bt1[:])
            nc.sync.dma_start(g3_sb, g3[:])
            nc.sync.dma_start(bt3_sb, bt3[:])

            # PE HAM warm-up: a dense burst of matmuls on the (tiny) w1
            # tile while the big xt DMAs stream in.
            for i in range(24):
                pw = psa.tile([D, NW], f32, tag="pp")
                nc.tensor.matmul(pw[:, 0:NW // 2], lhsT=w1_sb[:, 0:D],
                                 rhs=w1_sb[:, 0:NW // 2], start=True, stop=True)
                nc.tensor.matmul(pw[:, NW // 2:NW], lhsT=w1_sb[:, 0:D],
                                 rhs=w1_sb[:, NW // 2:NW], start=True, stop=True)

            xu = []
            for p in range(P):
                t = xup.tile([D, BC], bf16, tag=f"xu{p}")
                nc.sync.dma_start(t, xt[p * D:(p + 1) * D, :])
                xu.append(t)

            stats1 = statp.tile([D, n_dve, 4, 6], f32)   # DVE blocks, 512-wide
            stats2 = statp.tile([D, P, 4, 6], f32)
            mv1 = statp.tile([D, n_dve, 2], f32)
            mv2 = statp.tile([D, P, 2], f32)
            sa = statp.tile([D, n_act, 2], f32)          # ACT-block sums
            qa = statp.tile([D, n_act, 2], f32)          # ACT-block sumsqs
            arpay1 = statp.tile([D, 2 * P], f32)
            arpay2 = statp.tile([D, 2 * P], f32)
            red1 = statp.tile([D, 2 * P], f32)
            red2 = statp.tile([D, 2 * P], f32)
            gath1 = statp.tile([D, NCORES, 2 * P], f32)
            gath2 = statp.tile([D, NCORES, 2 * P], f32)
            Mt = statp.tile([D, P], f32)
            Qt = statp.tile([D, P], f32)
            vt = statp.tile([D, P], f32)
            s1 = statp.tile([D, P], f32)
            t1 = statp.tile([D, P], f32)
            s3 = statp.tile([D, P], f32)
            t3 = statp.tile([D, P], f32)
            eps_sb = statp.tile([D, 1], f32)
            nc.vector.memset(eps_sb, EPS)

            def wcol(w_sb, p):
                return w_sb[:, p * D:(p + 1) * D]

            def all_gather(arpay, gath, red, tagn):
                agin = dram.tile([D, 2 * P], f32, tag=f"agin{tagn}")
                agout = dram.tile([NCORES * D, 2 * P], f32, tag=f"agout{tagn}")
                nc.sync.dma_start(agin, arpay)
                nc.gpsimd.collective_compute(
                    "AllGather", mybir.AluOpType.bypass,
                    replica_groups=[list(range(NCORES))],
                    ins=[agin.opt()], outs=[agout.opt()],
                )
                nc.sync.dma_start(gath, agout.rearrange("(r i) f -> i r f", r=NCORES))
                nc.vector.tensor_reduce(out=red, in_=gath[:].rearrange("i r f -> i f r"),
                                        axis=AX, op=add)

            def affine(red, g_sb, b_sb, s, t):
                # red[:, 0:P] = Σ_cores mean ; red[:, P:2P] = Σ_cores E[y²]
                nc.vector.tensor_scalar_mul(Mt, red[:, 0:P], 1.0 / NCORES)
                nc.vector.tensor_scalar_mul(Qt, red[:, P:2 * P], 1.0 / NCORES)
                nc.vector.tensor_tensor(vt, Mt, Mt, op=mult)
                nc.vector.tensor_tensor(vt, Qt, vt, op=subtract)          # global var
                nc.scalar.activation(out=vt, in_=vt, func=Sqrt, bias=eps_sb)
                nc.vector.reciprocal(vt, vt)                              # rstd
                nc.vector.tensor_tensor(s, g_sb, vt, op=mult)
                nc.vector.tensor_tensor(t, Mt, s, op=mult)
                nc.vector.tensor_tensor(t, b_sb, t, op=subtract)          # beta - M*s

            # ---- Stage A: per-core stats of y1 = x @ W1 ----
            for p in range(P):
                j = None
                if p in ACT_BLOCKS:
                    j = ACT_BLOCKS.index(p)
                else:
                    j = DVE_BLOCKS.index(p)
                pool = psa if p % 2 == 0 else psb
                for h in range(NH):
                    ps = pool.tile([D, NW], f32, tag="pp" if pool is psa else "qq")
                    for q in range(2):
                        qs = slice(q * (NW // 2), (q + 1) * (NW // 2))
                        nc.tensor.matmul(ps[:, qs], lhsT=wcol(w1_sb, p),
                                         rhs=xu[p][:, h * NW + q * (NW // 2):
                                                   h * NW + (q + 1) * (NW // 2)],
                                         start=True, stop=True)
                    if p in ACT_BLOCKS:
                        scr = scrp.tile([D, NW], bf16, tag="scr")
                        nc.scalar.activation(out=scr, in_=ps, func=Copy,
                                             accum_out=sa[:, j, h:h + 1])
                        nc.scalar.activation(out=scr, in_=ps, func=Square,
                                             accum_out=qa[:, j, h:h + 1])
                    else:
                        nc.vector.bn_stats(out=stats1[:, j, 2 * h], in_=ps[:, 0:NW // 2])
                        nc.vector.bn_stats(out=stats1[:, j, 2 * h + 1], in_=ps[:, NW // 2:NW])
                if p not in ACT_BLOCKS:
                    nc.vector.bn_aggr(out=mv1[:, j], in_=stats1[:, j])

            # payload (grouped column order: DVE blocks then ACT blocks)
            nc.vector.tensor_copy(arpay1[:, 0:n_dve], mv1[:, :, 0])
            nc.vector.tensor_tensor(arpay1[:, P:P + n_dve], mv1[:, :, 0], mv1[:, :, 0], op=mult)
            nc.vector.tensor_tensor(arpay1[:, P:P + n_dve], arpay1[:, P:P + n_dve],
                                    mv1[:, :, 1], op=add)
            nc.vector.tensor_reduce(out=arpay1[:, n_dve:P], in_=sa[:], axis=AX, op=add)
            nc.vector.tensor_reduce(out=arpay1[:, P + n_dve:2 * P], in_=qa[:], axis=AX, op=add)
            nc.vector.tensor_scalar_mul(arpay1[:, n_dve:P], arpay1[:, n_dve:P], 1.0 / BC)
            nc.vector.tensor_scalar_mul(arpay1[:, P + n_dve:2 * P],
                                        arpay1[:, P + n_dve:2 * P], 1.0 / BC)

            all_gather(arpay1, gath1, red1, 1)
            # keep the PE HAM warm through the collective gap (slot reuse of
            # the "pp" pool orders these after stage A's matmuls)
            for i in range(20):
                pw = psa.tile([D, NW], f32, tag="pp")
                nc.tensor.matmul(pw[:, 0:NW // 2], lhsT=w1_sb[:, 0:D],
                                 rhs=w1_sb[:, 0:NW // 2], start=True, stop=True)
                nc.tensor.matmul(pw[:, NW // 2:NW], lhsT=w1_sb[:, 0:D],
                                 rhs=w1_sb[:, NW // 2:NW], start=True, stop=True)
            affine(red1, g1_sb, bt1_sb, s1, t1)   # grouped col order

            # ---- Stage B: o1 = tanh(s1·y1 + t1); u = o1 @ W2 + x ----
            for p in range(P):
                c1 = COL1[p]
                o1 = o1p.tile([D, BC], bf16, tag="o1")
                pss = []
                for h in range(NH):
                    ps = psa.tile([D, NW], f32, tag="pp")
                    pss.append(ps)
                    for q in range(2):
                        nc.tensor.matmul(ps[:, q * (NW // 2):(q + 1) * (NW // 2)],
                                         lhsT=wcol(w1_sb, p),
                                         rhs=xu[p][:, h * NW + q * (NW // 2):
                                                   h * NW + (q + 1) * (NW // 2)],
                                         start=True, stop=True)
                for h in range(NH):
                    hs = slice(h * NW, (h + 1) * NW)
                    nc.scalar.activation(out=o1[:, hs], in_=pss[h], func=Tanh,
                                         bias=t1[:, c1:c1 + 1], scale=s1[:, c1:c1 + 1])
                # one LDW of W2 for all four halves, then one LDW of identity
                pus = [psb.tile([D, NW], f32, tag="qq", name=f"pu{h}") for h in range(NH)]
                for h in range(NH):
                    for q in range(2):
                        gsl = slice(h * NW + q * (NW // 2), h * NW + (q + 1) * (NW // 2))
                        nc.tensor.matmul(pus[h][:, q * (NW // 2):(q + 1) * (NW // 2)],
                                         lhsT=wcol(w2_sb, p), rhs=o1[:, gsl],
                                         start=True, stop=False)
                for h in range(NH):
                    for q in range(2):
                        gsl = slice(h * NW + q * (NW // 2), h * NW + (q + 1) * (NW // 2))
                        nc.tensor.matmul(pus[h][:, q * (NW // 2):(q + 1) * (NW // 2)],
                                         lhsT=id_sb, rhs=xu[p][:, gsl],
                                         start=False, stop=True)
                for h in range(NH):
                    hs = slice(h * NW, (h + 1) * NW)
                    if p % 3 == 2:
                        nc.vector.tensor_copy(out=xu[p][:, hs], in_=pus[h])
                    else:
                        nc.scalar.activation(out=xu[p][:, hs], in_=pus[h],
                                             func=Copy)   # u overwrites x
                    nc.vector.bn_stats(out=stats2[:, p, 2 * h],
                                       in_=xu[p][:, h * NW:h * NW + NW // 2])
                    nc.vector.bn_stats(out=stats2[:, p, 2 * h + 1],
                                       in_=xu[p][:, h * NW + NW // 2:(h + 1) * NW])
                nc.vector.bn_aggr(out=mv2[:, p], in_=stats2[:, p])

            nc.vector.tensor_copy(arpay2[:, 0:P], mv2[:, :, 0])
            nc.vector.tensor_tensor(arpay2[:, P:2 * P], mv2[:, :, 0], mv2[:, :, 0], op=mult)
            nc.vector.tensor_tensor(arpay2[:, P:2 * P], arpay2[:, P:2 * P],
                                    mv2[:, :, 1], op=add)

            all_gather(arpay2, gath2, red2, 2)
            affine(red2, g3_sb, bt3_sb, s3, t3)   # natural col order

            # ---- Stage C: out = tanh(s3·u + t3) ----
            for p in range(P):
                of = ofp.tile([D, BC], bf16, tag="of")
                nc.scalar.activation(out=of, in_=xu[p], func=Tanh,
                                     bias=t3[:, p:p + 1], scale=s3[:, p:p + 1])
                nc.sync.dma_start(out[p * D:(p + 1) * D, :], of)

    return nc


def _get_nc():
    if "nc" not in _state:
        _install_tile_drain_patch()
        _install_ldw_opt_patch()
        _install_ntff_hook()
        _state["nc"] = _build()
    return _state["nc"]


def kernel(x, weights1, bias1, weights2, bias2, gamma1, beta1, gamma3, beta3):
    from concourse.bass_utils import run_bass_kernel_spmd

    x = np.asarray(x, dtype=np.float32)
    w1 = np.asarray(weights1, dtype=np.float32)
    w2 = np.asarray(weights2, dtype=np.float32)
    gamma1 = np.asarray(gamma1, dtype=np.float32)
    beta1 = np.asarray(beta1, dtype=np.float32)
    gamma3 = np.asarray(gamma3, dtype=np.float32)
    beta3 = np.asarray(beta3, dtype=np.float32)

    nc = _get_nc()

    xT = np.ascontiguousarray(x.T).astype(_BF16)            # [F, B]
    w1h = np.ascontiguousarray(w1.transpose(1, 0, 2).reshape(D, F)).astype(_BF16)
    w2h = np.ascontiguousarray(w2.transpose(1, 0, 2).reshape(D, F)).astype(_BF16)
    identh = np.eye(D, dtype=np.float32).astype(_BF16)
    perm = np.asarray(GROUPED)
    g1h = np.ascontiguousarray(gamma1.reshape(P, D).T[:, perm])
    bt1h = np.ascontiguousarray(beta1.reshape(P, D).T[:, perm])
    g3h = np.ascontiguousarray(gamma3.reshape(P, D).T)
    bt3h = np.ascontiguousarray(beta3.reshape(P, D).T)

    in_maps = []
    for cid in range(NCORES):
        in_maps.append({
            "xt": np.ascontiguousarray(xT[:, cid * BC:(cid + 1) * BC]),
            "w1": w1h, "w2": w2h, "ident": identh,
            "g1": g1h, "bt1": bt1h, "g3": g3h, "bt3": bt3h,
        })

    res = run_bass_kernel_spmd(nc, in_maps, core_ids=list(range(NCORES)))
    _state["last_exec_time_ns"] = res.exec_time_ns

    outT = np.empty((B, F), dtype=np.float32)
    for cid in range(NCORES):
        outT[cid * BC:(cid + 1) * BC, :] = res.results[cid]["out"].T.astype(np.float32)
    return outT


# revision 12
# speedup vs baseline: 2.2450x; 1.0597x over previous
"""Trainium2 Bass kernel for nn_Better_Transformer (block-diag MLP + BatchNorm + tanh ×2).

  o1 = tanh(BN(x @ blockdiag(w1) + b1))
  o3 = tanh(BN(o1 @ blockdiag(w2) + b2 + x))

Strategy (8 NeuronCores, data-parallel over the batch dim):
  - Each core owns 2048 of the 16384 rows; weights/BN params replicated.
  - Feature-major layout on chip ([128 features, rows]): BatchNorm
    reductions are free-dim reductions and matmuls stream rows as the
    moving operand (weights stationary), N=1024 bf16 moving tiles.
  - Host pre-transposes x to [F, B/8] bf16 per core; output returns
    feature-major bf16 and the host transposes/upcasts back.
  - bias1/bias2 cancel inside BatchNorm and never reach the device.
  - BN statistics: per-core (mean, E[y²]) per feature → 32 KB AllGather
    over the 8 cores → local reduce → global mean/var.  Stage-A stats
    are split between VectorE (bn_stats) and ScalarE (Copy/Square with
    accum_out) so both engines share the scan.
  - The residual (+x) is accumulated on the TensorEngine via an
    identity-matrix matmul into the same PSUM group as matmul2.
  - BN affine + tanh fuse into one ScalarEngine activation per tile
    (per-partition scale/bias APs).
  - y1 is recomputed in stage B instead of stored; u = o2+x overwrites
    the resident x blockwise (one 16 MB SBUF region holds x then u).
  - A warm-up burst of matmuls trips the PE HAM throttle to 2.4 GHz
    while the input DMAs are still in flight.
"""

import os
import sys
import types

import numpy as np
import ml_dtypes

B, F, P, D = 16384, 4096, 32, 128
NCORES = 8
BC = B // NCORES          # 2048 rows per core
NW = 1024                 # matmul moving-dim (bf16 allows 1024)
NH = BC // NW             # 2 wide chunks per block row-range
EPS = 1e-5

# Stage-A engine split: these blocks' stats run on ScalarE (accum_out),
# the rest on VectorE (bn_stats).  ~13/32 balances 2×FD1024 ACT ops
# against 4×FD512 bn_stats.
ACT_BLOCKS = [0, 3, 6, 9, 12, 15, 18, 21, 24, 27, 30]
DVE_BLOCKS = [p for p in range(P) if p not in ACT_BLOCKS]
# Sync-1 runs as two half-batch AllGathers (blocks 0-15 gathered while
# blocks 16-31 are still computing).  Payload column order groups by
# (half, engine) so every payload write is a contiguous batched op.
DVE_A = [p for p in DVE_BLOCKS if p < 16]
ACT_A = [p for p in ACT_BLOCKS if p < 16]
DVE_B = [p for p in DVE_BLOCKS if p >= 16]
ACT_B = [p for p in ACT_BLOCKS if p >= 16]
GROUPED = DVE_A + ACT_A + DVE_B + ACT_B
COL1 = {p: i for i, p in enumerate(GROUPED)}
NDA, NAA, NDB, NAB = len(DVE_A), len(ACT_A), len(DVE_B), len(ACT_B)

_BF16 = ml_dtypes.bfloat16

_state: dict = {}


def _install_ldw_opt_patch():
    """bass hardcodes --enable-ldw-opt=false; walrus's own default is
    true.  Re-enable it (BASS_LDW_OPT=0 reverts) so repeated-lhsT matmul
    runs don't reload the PE weight array every instruction."""
    if _state.get("ldw_patched") or os.environ.get("BASS_LDW_OPT", "0") != "1":
        return
    _state["ldw_patched"] = True
    import concourse.bass_utils as bu
    real = bu.run_command

    def wrapper(argv, **kw):
        argv = ["--enable-ldw-opt=true" if a == "--enable-ldw-opt=false" else a
                for a in argv]
        return real(argv, **kw)

    bu.run_command = wrapper


def _install_tile_drain_patch():
    """This walrus build rejects >1 sem wait per instruction ("Too many
    sync wait commands" in setupSyncWait).  1) split the end-of-kernel
    drain waits across single-wait NOPs; 2) after assign_waits, hoist
    extra per-instruction waits onto nofuse NOPs."""
    if _state.get("patched"):
        return
    _state["patched"] = True
    import concourse.mybir as mybir
    import concourse.tile as tile_mod
    from concourse.tile import TileContext
    from concourse.vector_clock import ScopedClock, VectorClock

    def _drain_and_barrier(self, tick_clock, wait_clock):
        gc = tick_clock.global_clock
        for i in range(len(gc)):
            if gc[i] > 0:
                c = VectorClock()
                c.require_at_least(i, gc[i])
                nop = self.nc.sync.nop(nofuse=True, hint="tile_exit_wait")
                wait_clock.add_sem_waits(nop.ins, ScopedClock({None: c}))
        self.nc.sync.drain()
        self.nc.all_engine_barrier()
        assert self.sems is not None
        popped = self.nc._tile_sem_poison_stack.pop()
        assert popped is self._sem_poison
        self.nc.clear_and_free_semaphores(list(self.sems.allocated().values()))
        self.nc.all_engine_barrier()

    TileContext._drain_and_barrier = _drain_and_barrier

    _RealWait = tile_mod.TileClockWait

    class _WaitSplitClockWait:
        def __init__(self, tc, ordered):
            self._w = _RealWait(tc, ordered)
            self._tc = tc
            self._ordered = ordered

        def assign_waits(self, bb_name):
            r = self._w.assign_waits(bb_name)
            nc = self._tc.nc
            for insts in self._ordered.values():
                out = []
                for inst in insts:
                    si = inst.sync_info
                    if si is not None and si.on_wait and len(si.on_wait) > 1:
                        waits = list(si.on_wait)
                        for w in waits[:-1]:
                            nop = mybir.InstNoOp(
                                name=nc.get_next_instruction_name(),
                                engine=inst.engine, ins=[], outs=[],
                            )
                            nop.bass_nofuse = True
                            nop.sync_info = mybir.SyncInfo(on_wait=[w], on_update=[])
                            out.append(nop)
                        si.on_wait = [waits[-1]]
                    out.append(inst)
                insts[:] = out
            return r

        def __getattr__(self, k):
            return getattr(self._w, k)

    tile_mod.TileClockWait = _WaitSplitClockWait


def _install_ntff_hook():
    """Optional: lets BASS_TRACE=1 produce an NTFF profile under axon when
    the image's antenv lacks axon_hooks.  Safe no-op on any failure."""
    if "antenv.axon_hooks" in sys.modules:
        return
    try:
        import contextlib
        import ctypes

        so_path = "/opt/axon/libaxon_pjrt.so"
        if not os.path.exists(so_path):
            return
        lib = ctypes.CDLL(so_path)
        if not hasattr(lib, "axon_start_nrt_profile"):
            return
        lib.axon_start_nrt_profile.argtypes = [ctypes.POINTER(ctypes.c_int64), ctypes.c_size_t]
        lib.axon_start_nrt_profile.restype = ctypes.c_int64
        lib.axon_stop_nrt_profile.argtypes = [ctypes.c_char_p]
        lib.axon_stop_nrt_profile.restype = ctypes.c_int64

        @contextlib.contextmanager
        def _hook(output_dir, device_ids):
            import jax
            jax.devices()
            if device_ids:
                ids = (ctypes.c_int64 * len(device_ids))(*device_ids)
                rc = lib.axon_start_nrt_profile(ids, len(device_ids))
            else:
                rc = lib.axon_start_nrt_profile(None, 0)
            if rc != 0:
                raise RuntimeError(f"axon_start_nrt_profile rc={rc}")
            try:
                yield
            finally:
                n = lib.axon_stop_nrt_profile(str(output_dir).encode())
                if n <= 0:
                    print(f"ntff profile: {n} files written", file=sys.stderr)

        mod = types.ModuleType("antenv.axon_hooks")
        mod.get_axon_ntff_profile_hook = lambda: _hook
        mod.set_axon_ntff_profile_hook = lambda h: None
        sys.modules["antenv.axon_hooks"] = mod
    except Exception:
        pass


def _build():
    import concourse.bass as bass
    import concourse.mybir as mybir
    import concourse.tile as tile

    f32 = mybir.dt.float32
    bf16 = mybir.dt.bfloat16
    Tanh = mybir.ActivationFunctionType.Tanh
    Sqrt = mybir.ActivationFunctionType.Sqrt
    Copy = mybir.ActivationFunctionType.Copy
    Square = mybir.ActivationFunctionType.Square
    mult = mybir.AluOpType.mult
    add = mybir.AluOpType.add
    subtract = mybir.AluOpType.subtract
    AX = mybir.AxisListType.X

    nc = bass.Bass(trn_type="TRN2", num_devices=NCORES)

    xt = nc.dram_tensor("xt", [F, BC], bf16, kind="ExternalInput")
    w1 = nc.dram_tensor("w1", [D, F], bf16, kind="ExternalInput")
    w2 = nc.dram_tensor("w2", [D, F], bf16, kind="ExternalInput")
    ident = nc.dram_tensor("ident", [D, D], bf16, kind="ExternalInput")
    g1 = nc.dram_tensor("g1", [D, P], f32, kind="ExternalInput")   # grouped col order
    bt1 = nc.dram_tensor("bt1", [D, P], f32, kind="ExternalInput")  # grouped col order
    g3 = nc.dram_tensor("g3", [D, P], f32, kind="ExternalInput")   # natural order
    bt3 = nc.dram_tensor("bt3", [D, P], f32, kind="ExternalInput")
    out = nc.dram_tensor("out", [F, BC], bf16, kind="ExternalOutput")

    n_act = len(ACT_BLOCKS)
    n_dve = len(DVE_BLOCKS)

    with tile.TileContext(nc) as tc:
        with (
            tc.tile_pool(name="const", bufs=1) as const,
            tc.tile_pool(name="xup", bufs=1) as xup,
            tc.tile_pool(name="stat", bufs=1) as statp,
            tc.tile_pool(name="o1p", bufs=2) as o1p,
            tc.tile_pool(name="scrp", bufs=2) as scrp,
            tc.tile_pool(name="ofp", bufs=4) as ofp,
            tc.tile_pool(name="psa", bufs=2, space="PSUM") as psa,
            tc.tile_pool(name="psb", bufs=2, space="PSUM") as psb,
            tc.tile_pool(name="dram", bufs=1, space="DRAM") as dram,
        ):
            w1_sb = const.tile([D, F], bf16)
            w2_sb = const.tile([D, F], bf16)
            id_sb = const.tile([D, D], bf16)
            g1_sb = const.tile([D, P], f32)
            bt1_sb = const.tile([D, P], f32)
            g3_sb = const.tile([D, P], f32)
            bt3_sb = const.tile([D, P], f32)
            nc.sync.dma_start(w1_sb, w1[:])
            nc.sync.dma_start(w2_sb, w2[:])
            nc.sync.dma_start(id_sb, ident[:])
            nc.sync.dma_start(g1_sb, g1[:])
            nc.sync.dma_start(bt1_sb, bt1[:])
            nc.sync.dma_start(g3_sb, g3[:])
            nc.sync.dma_start(bt3_sb, bt3[:])

            # PE HAM warm-up: a dense burst of matmuls on the (tiny) w1
            # tile while the big xt DMAs stream in.
            for i in range(24):
                pw = psa.tile([D, NW], f32, tag="pp")
                nc.tensor.matmul(pw[:, 0:NW // 2], lhsT=w1_sb[:, 0:D],
                                 rhs=w1_sb[:, 0:NW // 2], start=True, stop=True)
                nc.tensor.matmul(pw[:, NW // 2:NW], lhsT=w1_sb[:, 0:D],
                                 rhs=w1_sb[:, NW // 2:NW], start=True, stop=True)

            xu = []
            for p in range(P):
                t = xup.tile([D, BC], bf16, tag=f"xu{p}")
                nc.sync.dma_start(t, xt[p * D:(p + 1) * D, :])
                xu.append(t)

            stats1 = statp.tile([D, n_dve, 4, 6], f32)   # DVE blocks, 512-wide
            stats2 = statp.tile([D, P, 4, 6], f32)
            mv1 = statp.tile([D, n_dve, 2], f32)
            mv2 = statp.tile([D, P, 2], f32)
            sa = statp.tile([D, n_act, 2], f32)          # ACT-block sums
            qa = statp.tile([D, n_act, 2], f32)          # ACT-block sumsqs
            arpay1a = statp.tile([D, P], f32)
            arpay1b = statp.tile([D, P], f32)
            arpay2 = statp.tile([D, 2 * P], f32)
            red1a = statp.tile([D, P], f32)
            red1b = statp.tile([D, P], f32)
            red2 = statp.tile([D, 2 * P], f32)
            gath1a = statp.tile([D, NCORES, P], f32)
            gath1b = statp.tile([D, NCORES, P], f32)
            gath2 = statp.tile([D, NCORES, 2 * P], f32)
            Mt = statp.tile([D, P], f32)
            Qt = statp.tile([D, P], f32)
            vt = statp.tile([D, P], f32)
            s1 = statp.tile([D, P], f32)
            t1 = statp.tile([D, P], f32)
            s3 = statp.tile([D, P], f32)
            t3 = statp.tile([D, P], f32)
            eps_sb = statp.tile([D, 1], f32)
            nc.vector.memset(eps_sb, EPS)

            def wcol(w_sb, p):
                return w_sb[:, p * D:(p + 1) * D]

            def all_gather(arpay, gath, red, tagn):
                npay = arpay.shape[-1]
                agin = dram.tile([D, npay], f32, tag=f"agin{tagn}", name=f"agin{tagn}")
                agout = dram.tile([NCORES * D, npay], f32, tag=f"agout{tagn}",
                                  name=f"agout{tagn}")
                nc.sync.dma_start(agin, arpay)
                nc.gpsimd.collective_compute(
                    "AllGather", mybir.AluOpType.bypass,
                    replica_groups=[list(range(NCORES))],
                    ins=[agin.opt()], outs=[agout.opt()],
                )
                nc.sync.dma_start(gath, agout.rearrange("(r i) f -> i r f", r=NCORES))
                nc.vector.tensor_reduce(out=red, in_=gath[:].rearrange("i r f -> i f r"),
                                        axis=AX, op=add)

            def affine(red, g_sb, b_sb, s, t):
                # red[:, 0:P] = Σ_cores mean ; red[:, P:2P] = Σ_cores E[y²]
                nc.vector.tensor_scalar_mul(Mt, red[:, 0:P], 1.0 / NCORES)
                nc.vector.tensor_scalar_mul(Qt, red[:, P:2 * P], 1.0 / NCORES)
                nc.vector.tensor_tensor(vt, Mt, Mt, op=mult)
                nc.vector.tensor_tensor(vt, Qt, vt, op=subtract)          # global var
                nc.scalar.activation(out=vt, in_=vt, func=Sqrt, bias=eps_sb)
                nc.vector.reciprocal(vt, vt)                              # rstd
                nc.vector.tensor_tensor(s, g_sb, vt, op=mult)
                nc.vector.tensor_tensor(t, Mt, s, op=mult)
                nc.vector.tensor_tensor(t, b_sb, t, op=subtract)          # beta - M*s

            # ---- Stage A: per-core stats of y1 = x @ W1 ----
            for p in range(P):
                j = None
                if p in ACT_BLOCKS:
                    j = ACT_BLOCKS.index(p)
                else:
                    j = DVE_BLOCKS.index(p)
                pool = psa if p % 2 == 0 else psb
                for h in range(NH):
                    ps = pool.tile([D, NW], f32, tag="pp" if pool is psa else "qq")
                    for q in range(2):
                        qs = slice(q * (NW // 2), (q + 1) * (NW // 2))
                        nc.tensor.matmul(ps[:, qs], lhsT=wcol(w1_sb, p),
                                         rhs=xu[p][:, h * NW + q * (NW // 2):
                                                   h * NW + (q + 1) * (NW // 2)],
                                         start=True, stop=True)
                    if p in ACT_BLOCKS:
                        scr = scrp.tile([D, NW], bf16, tag="scr")
                        nc.scalar.activation(out=scr, in_=ps, func=Copy,
                                             accum_out=sa[:, j, h:h + 1])
                        nc.scalar.activation(out=scr, in_=ps, func=Square,
                                             accum_out=qa[:, j, h:h + 1])
                    else:
                        nc.vector.bn_stats(out=stats1[:, j, 2 * h], in_=ps[:, 0:NW // 2])
                        nc.vector.bn_stats(out=stats1[:, j, 2 * h + 1], in_=ps[:, NW // 2:NW])
                if p not in ACT_BLOCKS:
                    nc.vector.bn_aggr(out=mv1[:, j], in_=stats1[:, j])

                if p == 15:
                    # half-a payload: [DVE_A means | ACT_A means | DVE_A E2 | ACT_A E2]
                    h2 = P // 2
                    nc.vector.tensor_copy(arpay1a[:, 0:NDA], mv1[:, 0:NDA, 0])
                    nc.vector.tensor_tensor(arpay1a[:, h2:h2 + NDA], mv1[:, 0:NDA, 0],
                                            mv1[:, 0:NDA, 0], op=mult)
                    nc.vector.tensor_tensor(arpay1a[:, h2:h2 + NDA],
                                            arpay1a[:, h2:h2 + NDA],
                                            mv1[:, 0:NDA, 1], op=add)
                    nc.vector.tensor_reduce(out=arpay1a[:, NDA:h2],
                                            in_=sa[:, 0:NAA], axis=AX, op=add)
                    nc.vector.tensor_reduce(out=arpay1a[:, h2 + NDA:P],
                                            in_=qa[:, 0:NAA], axis=AX, op=add)
                    nc.vector.tensor_scalar_mul(arpay1a[:, NDA:h2],
                                                arpay1a[:, NDA:h2], 1.0 / BC)
                    nc.vector.tensor_scalar_mul(arpay1a[:, h2 + NDA:P],
                                                arpay1a[:, h2 + NDA:P], 1.0 / BC)
                    all_gather(arpay1a, gath1a, red1a, "1a")

            # half-b payload
            h2 = P // 2
            nc.vector.tensor_copy(arpay1b[:, 0:NDB], mv1[:, NDA:n_dve, 0])
            nc.vector.tensor_tensor(arpay1b[:, h2:h2 + NDB], mv1[:, NDA:n_dve, 0],
                                    mv1[:, NDA:n_dve, 0], op=mult)
            nc.vector.tensor_tensor(arpay1b[:, h2:h2 + NDB], arpay1b[:, h2:h2 + NDB],
                                    mv1[:, NDA:n_dve, 1], op=add)
            nc.vector.tensor_reduce(out=arpay1b[:, NDB:h2], in_=sa[:, NAA:n_act],
                                    axis=AX, op=add)
            nc.vector.tensor_reduce(out=arpay1b[:, h2 + NDB:P], in_=qa[:, NAA:n_act],
                                    axis=AX, op=add)
            nc.vector.tensor_scalar_mul(arpay1b[:, NDB:h2], arpay1b[:, NDB:h2], 1.0 / BC)
            nc.vector.tensor_scalar_mul(arpay1b[:, h2 + NDB:P],
                                        arpay1b[:, h2 + NDB:P], 1.0 / BC)
            all_gather(arpay1b, gath1b, red1b, "1b")

            # keep the PE HAM warm through the collective gap (slot reuse of
            # the "pp" pool orders these after stage A's matmuls)
            for i in range(20):
                pw = psa.tile([D, NW], f32, tag="pp", name="pw")
                nc.tensor.matmul(pw[:, 0:NW // 2], lhsT=w1_sb[:, 0:D],
                                 rhs=w1_sb[:, 0:NW // 2], start=True, stop=True)
                nc.tensor.matmul(pw[:, NW // 2:NW], lhsT=w1_sb[:, 0:D],
                                 rhs=w1_sb[:, NW // 2:NW], start=True, stop=True)

            # affine from the two half-gathers (col order = GROUPED)
            nc.vector.tensor_scalar_mul(Mt[:, 0:h2], red1a[:, 0:h2], 1.0 / NCORES)
            nc.vector.tensor_scalar_mul(Mt[:, h2:P], red1b[:, 0:h2], 1.0 / NCORES)
            nc.vector.tensor_scalar_mul(Qt[:, 0:h2], red1a[:, h2:P], 1.0 / NCORES)
            nc.vector.tensor_scalar_mul(Qt[:, h2:P], red1b[:, h2:P], 1.0 / NCORES)
            nc.vector.tensor_tensor(vt, Mt, Mt, op=mult)
            nc.vector.tensor_tensor(vt, Qt, vt, op=subtract)
            nc.scalar.activation(out=vt, in_=vt, func=Sqrt, bias=eps_sb)
            nc.vector.reciprocal(vt, vt)
            nc.vector.tensor_tensor(s1, g1_sb, vt, op=mult)
            nc.vector.tensor_tensor(t1, Mt, s1, op=mult)
            nc.vector.tensor_tensor(t1, bt1_sb, t1, op=subtract)

            # ---- Stage B: o1 = tanh(s1·y1 + t1); u = o1 @ W2 + x ----
            for p in range(P):
                c1 = COL1[p]
                o1 = o1p.tile([D, BC], bf16, tag="o1")
                pss = []
                for h in range(NH):
                    ps = psa.tile([D, NW], f32, tag="pp")
                    pss.append(ps)
                    for q in range(2):
                        nc.tensor.matmul(ps[:, q * (NW // 2):(q + 1) * (NW // 2)],
                                         lhsT=wcol(w1_sb, p),
                                         rhs=xu[p][:, h * NW + q * (NW // 2):
                                                   h * NW + (q + 1) * (NW // 2)],
                                         start=True, stop=True)
                for h in range(NH):
                    hs = slice(h * NW, (h + 1) * NW)
                    nc.scalar.activation(out=o1[:, hs], in_=pss[h], func=Tanh,
                                         bias=t1[:, c1:c1 + 1], scale=s1[:, c1:c1 + 1])
                # one LDW of W2 for all four halves, then one LDW of identity
                pus = [psb.tile([D, NW], f32, tag="qq", name=f"pu{h}") for h in range(NH)]
                for h in range(NH):
                    for q in range(2):
                        gsl = slice(h * NW + q * (NW // 2), h * NW + (q + 1) * (NW // 2))
                        nc.tensor.matmul(pus[h][:, q * (NW // 2):(q + 1) * (NW // 2)],
                                         lhsT=wcol(w2_sb, p), rhs=o1[:, gsl],
                                         start=True, stop=False)
                for h in range(NH):
                    for q in range(2):
                        gsl = slice(h * NW + q * (NW // 2), h * NW + (q + 1) * (NW // 2))
                        nc.tensor.matmul(pus[h][:, q * (NW // 2):(q + 1) * (NW // 2)],
                                         lhsT=id_sb, rhs=xu[p][:, gsl],
                                         start=False, stop=True)
                for h in range(NH):
                    hs = slice(h * NW, (h + 1) * NW)
                    if p % 3 == 2:
                        nc.vector.tensor_copy(out=xu[p][:, hs], in_=pus[h])
                    else:
                        nc.scalar.activation(out=xu[p][:, hs], in_=pus[h],
                                             func=Copy)   # u overwrites x
                    nc.vector.bn_stats(out=stats2[:, p, 2 * h],
                                       in_=xu[p][:, h * NW:h * NW + NW // 2])
                    nc.vector.bn_stats(out=stats2[:, p, 2 * h + 1],
                                       in_=xu[p][:, h * NW + NW // 2:(h + 1) * NW])
                nc.vector.bn_aggr(out=mv2[:, p], in_=stats2[:, p])

            nc.vector.tensor_copy(arpay2[:, 0:P], mv2[:, :, 0])
            nc.vector.tensor_tensor(arpay2[:, P:2 * P], mv2[:, :, 0], mv2[:, :, 0], op=mult)
            nc.vector.tensor_tensor(arpay2[:, P:2 * P], arpay2[:, P:2 * P],
                                    mv2[:, :, 1], op=add)

            all_gather(arpay2, gath2, red2, 2)
            affine(red2, g3_sb, bt3_sb, s3, t3)   # natural col order

            # ---- Stage C: out = tanh(s3·u + t3) ----
            for p in range(P):
                of = ofp.tile([D, BC], bf16, tag="of")
                nc.scalar.activation(out=of, in_=xu[p], func=Tanh,
                                     bias=t3[:, p:p + 1], scale=s3[:, p:p + 1])
                nc.sync.dma_start(out[p * D:(p + 1) * D, :], of)

    return nc


def _get_nc():
    if "nc" not in _state:
        _install_tile_drain_patch()
        _install_ldw_opt_patch()
        _install_ntff_hook()
        _state["nc"] = _build()
    return _state["nc"]


def kernel(x, weights1, bias1, weights2, bias2, gamma1, beta1, gamma3, beta3):
    from concourse.bass_utils import run_bass_kernel_spmd

    x = np.asarray(x, dtype=np.float32)
    w1 = np.asarray(weights1, dtype=np.float32)
    w2 = np.asarray(weights2, dtype=np.float32)
    gamma1 = np.asarray(gamma1, dtype=np.float32)
    beta1 = np.asarray(beta1, dtype=np.float32)
    gamma3 = np.asarray(gamma3, dtype=np.float32)
    beta3 = np.asarray(beta3, dtype=np.float32)

    nc = _get_nc()

    xT = np.ascontiguousarray(x.T).astype(_BF16)            # [F, B]
    w1h = np.ascontiguousarray(w1.transpose(1, 0, 2).reshape(D, F)).astype(_BF16)
    w2h = np.ascontiguousarray(w2.transpose(1, 0, 2).reshape(D, F)).astype(_BF16)
    identh = np.eye(D, dtype=np.float32).astype(_BF16)
    perm = np.asarray(GROUPED)
    g1h = np.ascontiguousarray(gamma1.reshape(P, D).T[:, perm])
    bt1h = np.ascontiguousarray(beta1.reshape(P, D).T[:, perm])
    g3h = np.ascontiguousarray(gamma3.reshape(P, D).T)
    bt3h = np.ascontiguousarray(beta3.reshape(P, D).T)

    in_maps = []
    for cid in range(NCORES):
        in_maps.append({
            "xt": np.ascontiguousarray(xT[:, cid * BC:(cid + 1) * BC]),
            "w1": w1h, "w2": w2h, "ident": identh,
            "g1": g1h, "bt1": bt1h, "g3": g3h, "bt3": bt3h,
        })

    res = run_bass_kernel_spmd(nc, in_maps, core_ids=list(range(NCORES)))
    _state["last_exec_time_ns"] = res.exec_time_ns

    outT = np.empty((B, F), dtype=np.float32)
    for cid in range(NCORES):
        outT[cid * BC:(cid + 1) * BC, :] = res.results[cid]["out"].T.astype(np.float32)
    return outT


# revision 13
# speedup vs baseline: 2.2588x; 1.0061x over previous
"""Trainium2 Bass kernel for nn_Better_Transformer (block-diag MLP + BatchNorm + tanh ×2).

  o1 = tanh(BN(x @ blockdiag(w1) + b1))
  o3 = tanh(BN(o1 @ blockdiag(w2) + b2 + x))

Strategy (8 NeuronCores, data-parallel over the batch dim):
  - Each core owns 2048 of the 16384 rows; weights/BN params replicated.
  - Feature-major layout on chip ([128 features, rows]): BatchNorm
    reductions are free-dim reductions and matmuls stream rows as the
    moving operand (weights stationary), N=1024 bf16 moving tiles.
  - Host pre-transposes x to [F, B/8] bf16 per core; output returns
    feature-major bf16 and the host transposes/upcasts back.
  - bias1/bias2 cancel inside BatchNorm and never reach the device.
  - BN statistics: per-core (mean, E[y²]) per feature → 32 KB AllGather
    over the 8 cores → local reduce → global mean/var.  Stage-A stats
    are split between VectorE (bn_stats) and ScalarE (Copy/Square with
    accum_out) so both engines share the scan.
  - The residual (+x) is accumulated on the TensorEngine via an
    identity-matrix matmul into the same PSUM group as matmul2.
  - BN affine + tanh fuse into one ScalarEngine activation per tile
    (per-partition scale/bias APs).
  - y1 is recomputed in stage B instead of stored; u = o2+x overwrites
    the resident x blockwise (one 16 MB SBUF region holds x then u).
  - A warm-up burst of matmuls trips the PE HAM throttle to 2.4 GHz
    while the input DMAs are still in flight.
"""

import os
import sys
import types

import numpy as np
import ml_dtypes

B, F, P, D = 16384, 4096, 32, 128
NCORES = 8
BC = B // NCORES          # 2048 rows per core
NW = 1024                 # matmul moving-dim (bf16 allows 1024)
NH = BC // NW             # 2 wide chunks per block row-range
EPS = 1e-5

# Stage-A engine split: these blocks' stats run on ScalarE (accum_out),
# the rest on VectorE (bn_stats).  ~13/32 balances 2×FD1024 ACT ops
# against 4×FD512 bn_stats.
ACT_BLOCKS = [0, 3, 6, 9, 12, 15, 18, 21, 24, 27, 30]
DVE_BLOCKS = [p for p in range(P) if p not in ACT_BLOCKS]
# Sync-1 runs as two half-batch AllGathers (blocks 0-15 gathered while
# blocks 16-31 are still computing).  Payload column order groups by
# (half, engine) so every payload write is a contiguous batched op.
DVE_A = [p for p in DVE_BLOCKS if p < 16]
ACT_A = [p for p in ACT_BLOCKS if p < 16]
DVE_B = [p for p in DVE_BLOCKS if p >= 16]
ACT_B = [p for p in ACT_BLOCKS if p >= 16]
GROUPED = DVE_A + ACT_A + DVE_B + ACT_B
COL1 = {p: i for i, p in enumerate(GROUPED)}
NDA, NAA, NDB, NAB = len(DVE_A), len(ACT_A), len(DVE_B), len(ACT_B)

_BF16 = ml_dtypes.bfloat16

_state: dict = {}


def _install_ldw_opt_patch():
    """bass hardcodes --enable-ldw-opt=false; walrus's own default is
    true.  Re-enable it (BASS_LDW_OPT=0 reverts) so repeated-lhsT matmul
    runs don't reload the PE weight array every instruction."""
    if _state.get("ldw_patched") or os.environ.get("BASS_LDW_OPT", "0") != "1":
        return
    _state["ldw_patched"] = True
    import concourse.bass_utils as bu
    real = bu.run_command

    def wrapper(argv, **kw):
        argv = ["--enable-ldw-opt=true" if a == "--enable-ldw-opt=false" else a
                for a in argv]
        return real(argv, **kw)

    bu.run_command = wrapper


def _install_tile_drain_patch():
    """This walrus build rejects >1 sem wait per instruction ("Too many
    sync wait commands" in setupSyncWait).  1) split the end-of-kernel
    drain waits across single-wait NOPs; 2) after assign_waits, hoist
    extra per-instruction waits onto nofuse NOPs."""
    if _state.get("patched"):
        return
    _state["patched"] = True
    import concourse.mybir as mybir
    import concourse.tile as tile_mod
    from concourse.tile import TileContext
    from concourse.vector_clock import ScopedClock, VectorClock

    def _drain_and_barrier(self, tick_clock, wait_clock):
        gc = tick_clock.global_clock
        for i in range(len(gc)):
            if gc[i] > 0:
                c = VectorClock()
                c.require_at_least(i, gc[i])
                nop = self.nc.sync.nop(nofuse=True, hint="tile_exit_wait")
                wait_clock.add_sem_waits(nop.ins, ScopedClock({None: c}))
        self.nc.sync.drain()
        self.nc.all_engine_barrier()
        assert self.sems is not None
        popped = self.nc._tile_sem_poison_stack.pop()
        assert popped is self._sem_poison
        self.nc.clear_and_free_semaphores(list(self.sems.allocated().values()))
        self.nc.all_engine_barrier()

    TileContext._drain_and_barrier = _drain_and_barrier

    _RealWait = tile_mod.TileClockWait

    class _WaitSplitClockWait:
        def __init__(self, tc, ordered):
            self._w = _RealWait(tc, ordered)
            self._tc = tc
            self._ordered = ordered

        def assign_waits(self, bb_name):
            r = self._w.assign_waits(bb_name)
            nc = self._tc.nc
            for insts in self._ordered.values():
                out = []
                for inst in insts:
                    si = inst.sync_info
                    if si is not None and si.on_wait and len(si.on_wait) > 1:
                        waits = list(si.on_wait)
                        for w in waits[:-1]:
                            nop = mybir.InstNoOp(
                                name=nc.get_next_instruction_name(),
                                engine=inst.engine, ins=[], outs=[],
                            )
                            nop.bass_nofuse = True
                            nop.sync_info = mybir.SyncInfo(on_wait=[w], on_update=[])
                            out.append(nop)
                        si.on_wait = [waits[-1]]
                    out.append(inst)
                insts[:] = out
            return r

        def __getattr__(self, k):
            return getattr(self._w, k)

    tile_mod.TileClockWait = _WaitSplitClockWait


def _install_ntff_hook():
    """Optional: lets BASS_TRACE=1 produce an NTFF profile under axon when
    the image's antenv lacks axon_hooks.  Safe no-op on any failure."""
    if "antenv.axon_hooks" in sys.modules:
        return
    try:
        import contextlib
        import ctypes

        so_path = "/opt/axon/libaxon_pjrt.so"
        if not os.path.exists(so_path):
            return
        lib = ctypes.CDLL(so_path)
        if not hasattr(lib, "axon_start_nrt_profile"):
            return
        lib.axon_start_nrt_profile.argtypes = [ctypes.POINTER(ctypes.c_int64), ctypes.c_size_t]
        lib.axon_start_nrt_profile.restype = ctypes.c_int64
        lib.axon_stop_nrt_profile.argtypes = [ctypes.c_char_p]
        lib.axon_stop_nrt_profile.restype = ctypes.c_int64

        @contextlib.contextmanager
        def _hook(output_dir, device_ids):
            import jax
            jax.devices()
            if device_ids:
                ids = (ctypes.c_int64 * len(device_ids))(*device_ids)
                rc = lib.axon_start_nrt_profile(ids, len(device_ids))
            else:
                rc = lib.axon_start_nrt_profile(None, 0)
            if rc != 0:
                raise RuntimeError(f"axon_start_nrt_profile rc={rc}")
            try:
                yield
            finally:
                n = lib.axon_stop_nrt_profile(str(output_dir).encode())
                if n <= 0:
                    print(f"ntff profile: {n} files written", file=sys.stderr)

        mod = types.ModuleType("antenv.axon_hooks")
        mod.get_axon_ntff_profile_hook = lambda: _hook
        mod.set_axon_ntff_profile_hook = lambda h: None
        sys.modules["antenv.axon_hooks"] = mod
    except Exception:
        pass


def _build():
    import concourse.bass as bass
    import concourse.mybir as mybir
    import concourse.tile as tile

    f32 = mybir.dt.float32
    bf16 = mybir.dt.bfloat16
    Tanh = mybir.ActivationFunctionType.Tanh
    Sqrt = mybir.ActivationFunctionType.Sqrt
    Copy = mybir.ActivationFunctionType.Copy
    Square = mybir.ActivationFunctionType.Square
    mult = mybir.AluOpType.mult
    add = mybir.AluOpType.add
    subtract = mybir.AluOpType.subtract
    AX = mybir.AxisListType.X

    nc = bass.Bass(trn_type="TRN2", num_devices=NCORES)

    xt = nc.dram_tensor("xt", [F, BC], bf16, kind="ExternalInput")
    w1 = nc.dram_tensor("w1", [D, F], bf16, kind="ExternalInput")
    w2 = nc.dram_tensor("w2", [D, F], bf16, kind="ExternalInput")
    ident = nc.dram_tensor("ident", [D, D], bf16, kind="ExternalInput")
    g1 = nc.dram_tensor("g1", [D, P], f32, kind="ExternalInput")   # grouped col order
    bt1 = nc.dram_tensor("bt1", [D, P], f32, kind="ExternalInput")  # grouped col order
    g3 = nc.dram_tensor("g3", [D, P], f32, kind="ExternalInput")   # natural order
    bt3 = nc.dram_tensor("bt3", [D, P], f32, kind="ExternalInput")
    out = nc.dram_tensor("out", [F, BC], bf16, kind="ExternalOutput")

    n_act = len(ACT_BLOCKS)
    n_dve = len(DVE_BLOCKS)

    with tile.TileContext(nc) as tc:
        with (
            tc.tile_pool(name="const", bufs=1) as const,
            tc.tile_pool(name="xup", bufs=1) as xup,
            tc.tile_pool(name="stat", bufs=1) as statp,
            tc.tile_pool(name="o1p", bufs=2) as o1p,
            tc.tile_pool(name="scrp", bufs=2) as scrp,
            tc.tile_pool(name="ofp", bufs=4) as ofp,
            tc.tile_pool(name="psa", bufs=2, space="PSUM") as psa,
            tc.tile_pool(name="psb", bufs=2, space="PSUM") as psb,
            tc.tile_pool(name="dram", bufs=1, space="DRAM") as dram,
        ):
            w1_sb = const.tile([D, F], bf16)
            w2_sb = const.tile([D, F], bf16)
            id_sb = const.tile([D, D], bf16)
            g1_sb = const.tile([D, P], f32)
            bt1_sb = const.tile([D, P], f32)
            g3_sb = const.tile([D, P], f32)
            bt3_sb = const.tile([D, P], f32)
            nc.sync.dma_start(w1_sb, w1[:])
            nc.sync.dma_start(w2_sb, w2[:])
            nc.sync.dma_start(id_sb, ident[:])
            nc.sync.dma_start(g1_sb, g1[:])
            nc.sync.dma_start(bt1_sb, bt1[:])
            nc.sync.dma_start(g3_sb, g3[:])
            nc.sync.dma_start(bt3_sb, bt3[:])

            # PE HAM warm-up: a dense burst of matmuls on the (tiny) w1
            # tile while the big xt DMAs stream in.
            for i in range(24):
                pw = psa.tile([D, NW], f32, tag="pp")
                nc.tensor.matmul(pw[:, 0:NW // 2], lhsT=w1_sb[:, 0:D],
                                 rhs=w1_sb[:, 0:NW // 2], start=True, stop=True)
                nc.tensor.matmul(pw[:, NW // 2:NW], lhsT=w1_sb[:, 0:D],
                                 rhs=w1_sb[:, NW // 2:NW], start=True, stop=True)

            xu = []
            for p in range(P):
                t = xup.tile([D, BC], bf16, tag=f"xu{p}")
                nc.sync.dma_start(t, xt[p * D:(p + 1) * D, :])
                xu.append(t)

            stats1 = statp.tile([D, n_dve, 4, 6], f32)   # DVE blocks, 512-wide
            stats2 = statp.tile([D, P, 4, 6], f32)
            mv1 = statp.tile([D, n_dve, 2], f32)
            mv2 = statp.tile([D, P, 2], f32)
            sa = statp.tile([D, n_act, 2], f32)          # ACT-block sums
            qa = statp.tile([D, n_act, 2], f32)          # ACT-block sumsqs
            arpay1a = statp.tile([D, P], f32)
            arpay1b = statp.tile([D, P], f32)
            arpay2a = statp.tile([D, P], f32)
            arpay2b = statp.tile([D, P], f32)
            red1a = statp.tile([D, P], f32)
            red1b = statp.tile([D, P], f32)
            red2a = statp.tile([D, P], f32)
            red2b = statp.tile([D, P], f32)
            gath1a = statp.tile([D, NCORES, P], f32)
            gath1b = statp.tile([D, NCORES, P], f32)
            gath2a = statp.tile([D, NCORES, P], f32)
            gath2b = statp.tile([D, NCORES, P], f32)
            Mt = statp.tile([D, P], f32)
            Qt = statp.tile([D, P], f32)
            vt = statp.tile([D, P], f32)
            s1 = statp.tile([D, P], f32)
            t1 = statp.tile([D, P], f32)
            s3 = statp.tile([D, P], f32)
            t3 = statp.tile([D, P], f32)
            eps_sb = statp.tile([D, 1], f32)
            nc.vector.memset(eps_sb, EPS)

            def wcol(w_sb, p):
                return w_sb[:, p * D:(p + 1) * D]

            def all_gather(arpay, gath, red, tagn):
                npay = arpay.shape[-1]
                agin = dram.tile([D, npay], f32, tag=f"agin{tagn}", name=f"agin{tagn}")
                agout = dram.tile([NCORES * D, npay], f32, tag=f"agout{tagn}",
                                  name=f"agout{tagn}")
                nc.sync.dma_start(agin, arpay)
                nc.gpsimd.collective_compute(
                    "AllGather", mybir.AluOpType.bypass,
                    replica_groups=[list(range(NCORES))],
                    ins=[agin.opt()], outs=[agout.opt()],
                )
                nc.sync.dma_start(gath, agout.rearrange("(r i) f -> i r f", r=NCORES))
                nc.vector.tensor_reduce(out=red, in_=gath[:].rearrange("i r f -> i f r"),
                                        axis=AX, op=add)

            def affine(red, g_sb, b_sb, s, t):
                # red[:, 0:P] = Σ_cores mean ; red[:, P:2P] = Σ_cores E[y²]
                nc.vector.tensor_scalar_mul(Mt, red[:, 0:P], 1.0 / NCORES)
                nc.vector.tensor_scalar_mul(Qt, red[:, P:2 * P], 1.0 / NCORES)
                nc.vector.tensor_tensor(vt, Mt, Mt, op=mult)
                nc.vector.tensor_tensor(vt, Qt, vt, op=subtract)          # global var
                nc.scalar.activation(out=vt, in_=vt, func=Sqrt, bias=eps_sb)
                nc.vector.reciprocal(vt, vt)                              # rstd
                nc.vector.tensor_tensor(s, g_sb, vt, op=mult)
                nc.vector.tensor_tensor(t, Mt, s, op=mult)
                nc.vector.tensor_tensor(t, b_sb, t, op=subtract)          # beta - M*s

            # ---- Stage A: per-core stats of y1 = x @ W1 ----
            for p in range(P):
                j = None
                if p in ACT_BLOCKS:
                    j = ACT_BLOCKS.index(p)
                else:
                    j = DVE_BLOCKS.index(p)
                pool = psa if p % 2 == 0 else psb
                for h in range(NH):
                    ps = pool.tile([D, NW], f32, tag="pp" if pool is psa else "qq")
                    for q in range(2):
                        qs = slice(q * (NW // 2), (q + 1) * (NW // 2))
                        nc.tensor.matmul(ps[:, qs], lhsT=wcol(w1_sb, p),
                                         rhs=xu[p][:, h * NW + q * (NW // 2):
                                                   h * NW + (q + 1) * (NW // 2)],
                                         start=True, stop=True)
                    if p in ACT_BLOCKS:
                        scr = scrp.tile([D, NW], bf16, tag="scr")
                        nc.scalar.activation(out=scr, in_=ps, func=Copy,
                                             accum_out=sa[:, j, h:h + 1])
                        nc.scalar.activation(out=scr, in_=ps, func=Square,
                                             accum_out=qa[:, j, h:h + 1])
                    else:
                        nc.vector.bn_stats(out=stats1[:, j, 2 * h], in_=ps[:, 0:NW // 2])
                        nc.vector.bn_stats(out=stats1[:, j, 2 * h + 1], in_=ps[:, NW // 2:NW])
                if p not in ACT_BLOCKS:
                    nc.vector.bn_aggr(out=mv1[:, j], in_=stats1[:, j])

                if p == 15:
                    # half-a payload: [DVE_A means | ACT_A means | DVE_A E2 | ACT_A E2]
                    h2 = P // 2
                    nc.vector.tensor_copy(arpay1a[:, 0:NDA], mv1[:, 0:NDA, 0])
                    nc.vector.tensor_tensor(arpay1a[:, h2:h2 + NDA], mv1[:, 0:NDA, 0],
                                            mv1[:, 0:NDA, 0], op=mult)
                    nc.vector.tensor_tensor(arpay1a[:, h2:h2 + NDA],
                                            arpay1a[:, h2:h2 + NDA],
                                            mv1[:, 0:NDA, 1], op=add)
                    nc.vector.tensor_reduce(out=arpay1a[:, NDA:h2],
                                            in_=sa[:, 0:NAA], axis=AX, op=add)
                    nc.vector.tensor_reduce(out=arpay1a[:, h2 + NDA:P],
                                            in_=qa[:, 0:NAA], axis=AX, op=add)
                    nc.vector.tensor_scalar_mul(arpay1a[:, NDA:h2],
                                                arpay1a[:, NDA:h2], 1.0 / BC)
                    nc.vector.tensor_scalar_mul(arpay1a[:, h2 + NDA:P],
                                                arpay1a[:, h2 + NDA:P], 1.0 / BC)
                    all_gather(arpay1a, gath1a, red1a, "1a")

            # half-b payload
            h2 = P // 2
            nc.vector.tensor_copy(arpay1b[:, 0:NDB], mv1[:, NDA:n_dve, 0])
            nc.vector.tensor_tensor(arpay1b[:, h2:h2 + NDB], mv1[:, NDA:n_dve, 0],
                                    mv1[:, NDA:n_dve, 0], op=mult)
            nc.vector.tensor_tensor(arpay1b[:, h2:h2 + NDB], arpay1b[:, h2:h2 + NDB],
                                    mv1[:, NDA:n_dve, 1], op=add)
            nc.vector.tensor_reduce(out=arpay1b[:, NDB:h2], in_=sa[:, NAA:n_act],
                                    axis=AX, op=add)
            nc.vector.tensor_reduce(out=arpay1b[:, h2 + NDB:P], in_=qa[:, NAA:n_act],
                                    axis=AX, op=add)
            nc.vector.tensor_scalar_mul(arpay1b[:, NDB:h2], arpay1b[:, NDB:h2], 1.0 / BC)
            nc.vector.tensor_scalar_mul(arpay1b[:, h2 + NDB:P],
                                        arpay1b[:, h2 + NDB:P], 1.0 / BC)
            all_gather(arpay1b, gath1b, red1b, "1b")

            # keep the PE HAM warm through the collective gap (slot reuse of
            # the "pp" pool orders these after stage A's matmuls)
            for i in range(20):
                pw = psa.tile([D, NW], f32, tag="pp", name="pw")
                nc.tensor.matmul(pw[:, 0:NW // 2], lhsT=w1_sb[:, 0:D],
                                 rhs=w1_sb[:, 0:NW // 2], start=True, stop=True)
                nc.tensor.matmul(pw[:, NW // 2:NW], lhsT=w1_sb[:, 0:D],
                                 rhs=w1_sb[:, NW // 2:NW], start=True, stop=True)

            # affine from the two half-gathers (col order = GROUPED)
            nc.vector.tensor_scalar_mul(Mt[:, 0:h2], red1a[:, 0:h2], 1.0 / NCORES)
            nc.vector.tensor_scalar_mul(Mt[:, h2:P], red1b[:, 0:h2], 1.0 / NCORES)
            nc.vector.tensor_scalar_mul(Qt[:, 0:h2], red1a[:, h2:P], 1.0 / NCORES)
            nc.vector.tensor_scalar_mul(Qt[:, h2:P], red1b[:, h2:P], 1.0 / NCORES)
            nc.vector.tensor_tensor(vt, Mt, Mt, op=mult)
            nc.vector.tensor_tensor(vt, Qt, vt, op=subtract)
            nc.scalar.activation(out=vt, in_=vt, func=Sqrt, bias=eps_sb)
            nc.vector.reciprocal(vt, vt)
            nc.vector.tensor_tensor(s1, g1_sb, vt, op=mult)
            nc.vector.tensor_tensor(t1, Mt, s1, op=mult)
            nc.vector.tensor_tensor(t1, bt1_sb, t1, op=subtract)

            # ---- Stage B: o1 = tanh(s1·y1 + t1); u = o1 @ W2 + x ----
            for p in range(P):
                c1 = COL1[p]
                o1 = o1p.tile([D, BC], bf16, tag="o1")
                pss = []
                for h in range(NH):
                    ps = psa.tile([D, NW], f32, tag="pp")
                    pss.append(ps)
                    for q in range(2):
                        nc.tensor.matmul(ps[:, q * (NW // 2):(q + 1) * (NW // 2)],
                                         lhsT=wcol(w1_sb, p),
                                         rhs=xu[p][:, h * NW + q * (NW // 2):
                                                   h * NW + (q + 1) * (NW // 2)],
                                         start=True, stop=True)
                for h in range(NH):
                    hs = slice(h * NW, (h + 1) * NW)
                    nc.scalar.activation(out=o1[:, hs], in_=pss[h], func=Tanh,
                                         bias=t1[:, c1:c1 + 1], scale=s1[:, c1:c1 + 1])
                # one LDW of W2 for all four halves, then one LDW of identity
                pus = [psb.tile([D, NW], f32, tag="qq", name=f"pu{h}") for h in range(NH)]
                for h in range(NH):
                    for q in range(2):
                        gsl = slice(h * NW + q * (NW // 2), h * NW + (q + 1) * (NW // 2))
                        nc.tensor.matmul(pus[h][:, q * (NW // 2):(q + 1) * (NW // 2)],
                                         lhsT=wcol(w2_sb, p), rhs=o1[:, gsl],
                                         start=True, stop=False)
                for h in range(NH):
                    for q in range(2):
                        gsl = slice(h * NW + q * (NW // 2), h * NW + (q + 1) * (NW // 2))
                        nc.tensor.matmul(pus[h][:, q * (NW // 2):(q + 1) * (NW // 2)],
                                         lhsT=id_sb, rhs=xu[p][:, gsl],
                                         start=False, stop=True)
                for h in range(NH):
                    hs = slice(h * NW, (h + 1) * NW)
                    if p in (0, 4, 9, 13, 18, 22, 27):
                        nc.scalar.activation(out=xu[p][:, hs], in_=pus[h],
                                             func=Copy)   # u overwrites x
                    else:
                        nc.vector.tensor_copy(out=xu[p][:, hs], in_=pus[h])
                    nc.vector.bn_stats(out=stats2[:, p, 2 * h], in_=pus[h][:, 0:NW // 2])
                    nc.vector.bn_stats(out=stats2[:, p, 2 * h + 1], in_=pus[h][:, NW // 2:NW])
                nc.vector.bn_aggr(out=mv2[:, p], in_=stats2[:, p])

                if p == 15:
                    nc.vector.tensor_copy(arpay2a[:, 0:16], mv2[:, 0:16, 0])
                    nc.vector.tensor_tensor(arpay2a[:, 16:32], mv2[:, 0:16, 0],
                                            mv2[:, 0:16, 0], op=mult)
                    nc.vector.tensor_tensor(arpay2a[:, 16:32], arpay2a[:, 16:32],
                                            mv2[:, 0:16, 1], op=add)
                    all_gather(arpay2a, gath2a, red2a, "2a")

            nc.vector.tensor_copy(arpay2b[:, 0:16], mv2[:, 16:32, 0])
            nc.vector.tensor_tensor(arpay2b[:, 16:32], mv2[:, 16:32, 0],
                                    mv2[:, 16:32, 0], op=mult)
            nc.vector.tensor_tensor(arpay2b[:, 16:32], arpay2b[:, 16:32],
                                    mv2[:, 16:32, 1], op=add)
            all_gather(arpay2b, gath2b, red2b, "2b")

            def affine2(red, lo, hi, src_lo):
                w = hi - lo
                nc.vector.tensor_scalar_mul(Mt[:, lo:hi], red[:, src_lo:src_lo + w],
                                            1.0 / NCORES)
                nc.vector.tensor_scalar_mul(Qt[:, lo:hi],
                                            red[:, src_lo + 16:src_lo + 16 + w],
                                            1.0 / NCORES)
                nc.vector.tensor_tensor(vt[:, lo:hi], Mt[:, lo:hi], Mt[:, lo:hi], op=mult)
                nc.vector.tensor_tensor(vt[:, lo:hi], Qt[:, lo:hi], vt[:, lo:hi],
                                        op=subtract)
                nc.scalar.activation(out=vt[:, lo:hi], in_=vt[:, lo:hi], func=Sqrt,
                                     bias=eps_sb)
                nc.vector.reciprocal(vt[:, lo:hi], vt[:, lo:hi])
                nc.vector.tensor_tensor(s3[:, lo:hi], g3_sb[:, lo:hi], vt[:, lo:hi],
                                        op=mult)
                nc.vector.tensor_tensor(t3[:, lo:hi], Mt[:, lo:hi], s3[:, lo:hi], op=mult)
                nc.vector.tensor_tensor(t3[:, lo:hi], bt3_sb[:, lo:hi], t3[:, lo:hi],
                                        op=subtract)

            # ---- Stage C: out = tanh(s3·u + t3); half-a can start mid-stage-B ----
            affine2(red2a, 0, 16, 0)
            for p in range(16):
                of = ofp.tile([D, BC], bf16, tag="of", name="of")
                nc.scalar.activation(out=of, in_=xu[p], func=Tanh,
                                     bias=t3[:, p:p + 1], scale=s3[:, p:p + 1])
                nc.sync.dma_start(out[p * D:(p + 1) * D, :], of)
            affine2(red2b, 16, 32, 0)
            for p in range(16, P):
                of = ofp.tile([D, BC], bf16, tag="of", name="of")
                nc.scalar.activation(out=of, in_=xu[p], func=Tanh,
                                     bias=t3[:, p:p + 1], scale=s3[:, p:p + 1])
                nc.sync.dma_start(out[p * D:(p + 1) * D, :], of)

    return nc


def _get_nc():
    if "nc" not in _state:
        _install_tile_drain_patch()
        _install_ldw_opt_patch()
        _install_ntff_hook()
        _state["nc"] = _build()
    return _state["nc"]


def kernel(x, weights1, bias1, weights2, bias2, gamma1, beta1, gamma3, beta3):
    from concourse.bass_utils import run_bass_kernel_spmd

    x = np.asarray(x, dtype=np.float32)
    w1 = np.asarray(weights1, dtype=np.float32)
    w2 = np.asarray(weights2, dtype=np.float32)
    gamma1 = np.asarray(gamma1, dtype=np.float32)
    beta1 = np.asarray(beta1, dtype=np.float32)
    gamma3 = np.asarray(gamma3, dtype=np.float32)
    beta3 = np.asarray(beta3, dtype=np.float32)

    nc = _get_nc()

    xT = np.ascontiguousarray(x.T).astype(_BF16)            # [F, B]
    w1h = np.ascontiguousarray(w1.transpose(1, 0, 2).reshape(D, F)).astype(_BF16)
    w2h = np.ascontiguousarray(w2.transpose(1, 0, 2).reshape(D, F)).astype(_BF16)
    identh = np.eye(D, dtype=np.float32).astype(_BF16)
    perm = np.asarray(GROUPED)
    g1h = np.ascontiguousarray(gamma1.reshape(P, D).T[:, perm])
    bt1h = np.ascontiguousarray(beta1.reshape(P, D).T[:, perm])
    g3h = np.ascontiguousarray(gamma3.reshape(P, D).T)
    bt3h = np.ascontiguousarray(beta3.reshape(P, D).T)

    in_maps = []
    for cid in range(NCORES):
        in_maps.append({
            "xt": np.ascontiguousarray(xT[:, cid * BC:(cid + 1) * BC]),
            "w1": w1h, "w2": w2h, "ident": identh,
            "g1": g1h, "bt1": bt1h, "g3": g3h, "bt3": bt3h,
        })

    res = run_bass_kernel_spmd(nc, in_maps, core_ids=list(range(NCORES)))
    _state["last_exec_time_ns"] = res.exec_time_ns

    outT = np.empty((B, F), dtype=np.float32)
    for cid in range(NCORES):
        outT[cid * BC:(cid + 1) * BC, :] = res.results[cid]["out"].T.astype(np.float32)
    return outT
